# revision 36
# baseline (speedup 1.0000x reference)
"""STGCN fully on-device for 8 Trainium2 NeuronCores.

Data-parallel over batch (4 examples/core). Entire forward runs in ONE Bass
SPMD launch per call:
  - adjacency normalization on device
  - GCN1: A-first matmul (thin), transpose, W-matmul (block-diag packed,
    128-partition), BN1 stats via bn_stats + cross-core AllReduce, fused
    BN-apply+ReLU on ACT
  - GCN2: W-first matmul, DMA transpose, A-matmul (block-diag), DMA
    transpose back, BN2 (same path), residual add
  - LSTM0+LSTM1 pipelined scan: 129 slots, PSUM-preloaded xp/bias, 4+8
    matmuls/slot, fused sigmoid/tanh across both layers
  - FC head, output DMA

Layout glossary (per core, BL=4):
  jm in {0,1} node-half, node n = jm*32+m' ; channel o in [0,64)
  "L-layout":  [p=(jm,o)=128, (m'=32, b=4, t=128)]
  gates order: i, f, o, g  (PyTorch i,f,g,o permuted so sigmoid gates are
  adjacent)
"""
import os

import numpy as np
import ml_dtypes

import concourse.bass as bass
import concourse.tile as tile
from concourse import mybir
from concourse.bass_utils import run_bass_kernel_spmd

F32 = mybir.dt.float32
BF16 = mybir.dt.bfloat16
AF = mybir.ActivationFunctionType
ALU = mybir.AluOpType

B, N, T, FIN = 32, 64, 128, 9
GH, LH, NC_OUT = 64, 128, 16
NCORES = 8
BL = B // NCORES          # 4 examples per core
EPS = 1e-5
S_TOT = float(B * N * T)  # BN sample count (full batch)

# gate permutation: torch order i,f,g,o -> i,f,o,g
GPERM = np.concatenate([np.arange(0, 128), np.arange(128, 256),
                        np.arange(384, 512), np.arange(256, 384)])

LAST_EXEC_NS = None


def _bf16(a):
    return np.ascontiguousarray(np.asarray(a, np.float32).astype(ml_dtypes.bfloat16))


def _f32(a):
    return np.ascontiguousarray(np.asarray(a, np.float32))


# ---------------------------------------------------------------------------
# walrus wait-cap workaround (same as baseline)
# ---------------------------------------------------------------------------
def _split_excess_waits(nc):
    fix_id = 0
    for fn in nc.m.functions:
        for blk in fn.blocks:
            out = []
            changed = False
            for inst in blk.instructions:
                si = inst.sync_info
                waits = list(si.on_wait) if si and si.on_wait else []
                cap = 2 if isinstance(inst, mybir.InstEventSemaphore) else 1
                if len(waits) > cap:
                    extra, keep = waits[: len(waits) - cap], waits[len(waits) - cap:]
                    for w in extra:
                        nop = mybir.InstNoOp(name=f"waitfix-{fix_id}")
                        fix_id += 1
                        nop.engine = inst.engine
                        nop.sync_info = mybir.SyncInfo(on_wait=[w], on_update=[])
                        nop.debug = inst.debug
                        nc.register_instruction(nop, overwrite=True)
                        out.append(nop)
                    si.on_wait = keep
                    changed = True
                out.append(inst)
            if changed:
                blk.instructions = out
    return nc


# ---------------------------------------------------------------------------
# device program
# ---------------------------------------------------------------------------
def _build(debug=False):
    nc = bass.Bass(num_devices=NCORES)
    d = {}

    def din(name, shape, dt):
        d[name] = nc.dram_tensor(name, shape, dt, kind="ExternalInput")
        return d[name]

    xa = din("xa", [N, FIN, BL, T], BF16)          # x for A-mm1: [n,(f,b,t)]
    adj = din("adj", [N, N], F32)
    eye = din("eye", [N, N], F32)
    w1d = din("w1d", [2 * FIN, 128], BF16)         # blockdiag(w1,w1)
    w2d = din("w2d", [128, 128], BF16)             # blockdiag over jm of w2
    foldm = din("foldm", [128, 128], F32)          # mod-64 partition fold
    g1r = din("g1r", [128, 1], F32)
    b1r = din("b1r", [128, 1], F32)
    g2k = din("g2k", [128, 32], F32)               # gamma2 in (k,c2') layout
    b2k = din("b2k", [128, 32], F32)
    wih0 = din("wih0", [128, 32, 4, 128], BF16)    # [r=(jm,o), kchunk=m', kg, grow]
    wih0b = din("wih0b", [128, 32, 4, 128], BF16)  # [r=(k,m), kchunk=c2', kg, grow]
    fm2 = din("fm2", [128, 128], F32)              # blockdiag(ones64) fold
    whh0T = din("whh0T", [LH, 4 * LH], BF16)
    wih1T = din("wih1T", [LH, 4 * LH], BF16)
    whh1T = din("whh1T", [LH, 4 * LH], BF16)
    b0c = din("b0c", [128, 4], F32)                # bias0 per gate-chunk col
    b1t8 = din("b1t8", [128, 4, 8, BL], F32)       # L1 block-psum preload
    fc1T = din("fc1T", [LH, LH // 2], BF16)
    fc1b = din("fc1b", [LH // 2, 1], F32)
    fc2T = din("fc2T", [LH // 2, NC_OUT], BF16)
    fc2b = din("fc2b", [NC_OUT, 1], F32)
    y = nc.dram_tensor("y", [NC_OUT, BL], F32, kind="ExternalOutput")
    if debug:
        dbg_xp0 = nc.dram_tensor("dbg_xp0", [128, 4, BL, T], F32, kind="ExternalOutput")
        dbg_h0 = nc.dram_tensor("dbg_h0", [LH, BL], F32, kind="ExternalOutput")

    with tile.TileContext(nc) as tc:
        _prog(nc, tc, d, y,
              dbg=(None, dbg_xp0, dbg_h0) if debug else None)
    _split_excess_waits(nc)
    return nc


def _bn_block(nc, tc, wk, st, gr, br, dram, foldm_t, tag):
    """Aggregate precomputed bn_stats `st` [128,32,6] -> cross-core allreduce
    -> per-partition scale/bias [128,1] f32. Returns (sc, bs)."""
    mv = wk.tile([128, 2], F32)
    nc.vector.bn_aggr(mv[:], st[:])
    # local sums: n_loc = 32*BL*T per partition
    n_loc = float(32 * BL * T)
    sums = wk.tile([128, 2], F32)
    # sums[:,0] = mean*n_loc ; sums[:,1] = (var + mean^2)*n_loc
    msq = wk.tile([128, 1], F32)
    nc.vector.tensor_tensor(msq[:], mv[:, 0:1], mv[:, 0:1], ALU.mult)
    nc.vector.tensor_tensor(sums[:, 1:2], mv[:, 1:2], msq[:], ALU.add)
    nc.scalar.mul(sums[:, 0:1], mv[:, 0:1], n_loc)
    nc.scalar.mul(sums[:, 1:2], sums[:, 1:2], n_loc)
    # allreduce over 8 cores
    bi = dram.tile([128, 2], F32)
    bo = dram.tile([128, 2], F32)
    nc.sync.dma_start(bi[:], sums[:])
    nc.gpsimd.collective_compute(
        "AllReduce", ALU.add,
        replica_groups=[list(range(NCORES))],
        ins=[bi[:].opt()], outs=[bo[:].opt()])
    red = wk.tile([128, 2], F32)
    nc.sync.dma_start(red[:], bo[:])
    # fold jm-halves (mod-64) via PE: out[p,:] = sum_q foldm[q,p]*red[q,:]
    with tc.tile_pool(name=f"bnps{tag}", bufs=1, space="PSUM") as ps:
        pf = ps.tile([128, 2], F32)
        nc.tensor.matmul(pf[:], foldm_t[:], red[:], start=True, stop=True)
        tot = wk.tile([128, 2], F32)
        nc.scalar.mul(tot[:], pf[:], 1.0 / S_TOT)   # [mean, E[x^2]]
    var = wk.tile([128, 1], F32)
    m2 = wk.tile([128, 1], F32)
    nc.vector.tensor_tensor(m2[:], tot[:, 0:1], tot[:, 0:1], ALU.mult)
    nc.vector.tensor_tensor(var[:], tot[:, 1:2], m2[:], ALU.subtract)
    epst = wk.tile([128, 1], F32)
    nc.vector.memset(epst[:], EPS)
    sd = wk.tile([128, 1], F32)
    nc.scalar.activation(sd[:], var[:], AF.Sqrt, bias=epst[:, 0:1], scale=1.0)
    rs = wk.tile([128, 1], F32)
    nc.vector.reciprocal(rs[:], sd[:])
    sc = wk.tile([128, 1], F32)
    nc.vector.tensor_tensor(sc[:], rs[:], gr[:], ALU.mult)
    mscale = wk.tile([128, 1], F32)
    nc.vector.tensor_tensor(mscale[:], tot[:, 0:1], sc[:], ALU.mult)
    bs = wk.tile([128, 1], F32)
    nc.vector.tensor_tensor(bs[:], br[:], mscale[:], ALU.subtract)
    return sc, bs


def _prog(nc, tc, d, y, dbg=None):
    from contextlib import ExitStack
    stack = ExitStack()
    wk = stack.enter_context(tc.tile_pool(name="wk", bufs=1))       # persistent small
    dram = stack.enter_context(tc.tile_pool(name="drb", bufs=1, space="DRAM"))

    # ---------------- warm up the collective fabric (absorbs comm init +
    # launch skew off the BN1 allreduce's critical path)
    warm = wk.tile([128, 1], F32, name="warm")
    nc.vector.memset(warm[:], 0.0)
    wi = dram.tile([128, 1], F32, name="wi")
    wo = dram.tile([128, 1], F32, name="wo")
    nc.gpsimd.dma_start(wi[:], warm[:])
    nc.gpsimd.collective_compute(
        "AllReduce", ALU.add,
        replica_groups=[list(range(NCORES))],
        ins=[wi[:].opt()], outs=[wo[:].opt()])

    # ---------------- critical-path inputs first: xa + adjacency + w1d
    LEFT, RIGHT = "left", "right"
    pool_L = tc.alloc_tile_pool(name="pL", bufs=1, side=LEFT)    # x2 (long)
    pool_A = tc.alloc_tile_pool(name="pA", bufs=1, side=LEFT)    # xa, g1
    adj_t = wk.tile([N, N], F32, name="adj_t")
    eye_t = wk.tile([N, N], F32, name="eye_t")
    xa_t = pool_A.tile([N, FIN, BL, T], BF16, name="xa_t")
    w1d_t = wk.tile([2 * FIN, 128], BF16, name="w1d_t")
    with tc.high_priority():
        nc.sync.dma_start(xa_t[:, 0:3, :, :], d["xa"][:, 0:3, :, :])
        nc.scalar.dma_start(xa_t[:, 3:6, :, :], d["xa"][:, 3:6, :, :])
        nc.gpsimd.dma_start(xa_t[:, 6:9, :, :], d["xa"][:, 6:9, :, :])
        nc.sync.dma_start(adj_t[:], d["adj"][:])
        nc.sync.dma_start(eye_t[:], d["eye"][:])
        nc.sync.dma_start(w1d_t[:], d["w1d"][:])

    # ---------------- PE p-state warmup (reach 2.4GHz before GCN matmuls)
    wz = wk.tile([128, 512], BF16, name="wz")
    nc.vector.memset(wz[:], 0.0)
    with tc.tile_pool(name="wups", bufs=1, space="PSUM") as wps:
        wp = wps.tile([128, 512], F32, name="wp")
        for _ in range(8):
            nc.tensor.matmul(wp[:], wz[:, 0:128], wz[:], start=True, stop=True)

    # ---------------- remaining small constants (gpsimd queue: keeps the
    # sync ring free for xa/adj so A-mm1 can start early)
    foldm_t = wk.tile([128, 128], F32, name="foldm_t")
    nc.gpsimd.dma_start(foldm_t[:], d["foldm"][:])
    g1r = wk.tile([128, 1], F32, name="g1r")
    nc.gpsimd.dma_start(g1r[:], d["g1r"][:])
    b1r = wk.tile([128, 1], F32, name="b1r")
    nc.gpsimd.dma_start(b1r[:], d["b1r"][:])
    g2k = wk.tile([128, 32], F32, name="g2k")
    nc.gpsimd.dma_start(g2k[:], d["g2k"][:])
    b2k = wk.tile([128, 32], F32, name="b2k")
    nc.gpsimd.dma_start(b2k[:], d["b2k"][:])
    fm2_t = wk.tile([128, 128], F32, name="fm2_t")
    nc.gpsimd.dma_start(fm2_t[:], d["fm2"][:])
    w2d_t = wk.tile([128, 128], BF16, name="w2d_t")
    nc.gpsimd.dma_start(w2d_t[:], d["w2d"][:])

    a1 = wk.tile([N, N], F32)
    nc.vector.tensor_tensor(a1[:], adj_t[:], eye_t[:], ALU.add)
    deg = wk.tile([N, 1], F32)
    nc.vector.tensor_reduce(deg[:], a1[:], mybir.AxisListType.X, ALU.add)
    sdg = wk.tile([N, 1], F32)
    nc.scalar.activation(sdg[:], deg[:], AF.Sqrt)
    dinv = wk.tile([N, 1], F32)
    nc.vector.reciprocal(dinv[:], sdg[:])
    a2 = wk.tile([N, N], F32)
    nc.vector.tensor_scalar(a2[:], a1[:], dinv[:], None, ALU.mult)
    ones1 = wk.tile([1, N], F32)
    nc.vector.memset(ones1[:], 1.0)
    with tc.tile_pool(name="adjps", bufs=1, space="PSUM") as ps:
        pdt = ps.tile([1, N], F32)
        nc.tensor.transpose(pdt[:], dinv[:], eye_t[:])
        dT = wk.tile([1, N], F32)
        nc.scalar.copy(dT[:], pdt[:])
        pbc = ps.tile([N, N], F32)
        nc.tensor.matmul(pbc[:], ones1[:], dT[:], start=True, stop=True)
        ah = wk.tile([N, N], BF16)       # normalized adjacency, bf16
        nc.vector.tensor_tensor(ah[:], a2[:], pbc[:], ALU.mult)
    ahd = wk.tile([128, 128], BF16)      # blockdiag(ah, ah)
    nc.vector.memset(ahd[:], 0.0)
    nc.scalar.copy(ahd[0:64, 0:64], ah[:])
    nc.scalar.copy(ahd[64:128, 64:128], ah[:])

    # ---------------- LSTM weights; the two big 4MB tiles (wih0/wih0b) are
    # triggered AFTER A-mm1 below so their HBM traffic cannot delay xa/adj.
    wih0_t = wk.tile([128, 32, 4, 128], BF16, name="wih0_t")
    wih0b_t = wk.tile([128, 32, 4, 128], BF16, name="wih0b_t")
    whh0T_t = wk.tile([LH, 512], BF16, name="whh0T_t")
    nc.scalar.dma_start(whh0T_t[:], d["whh0T"][:])
    wih1T_t = wk.tile([LH, 512], BF16, name="wih1T_t")
    nc.scalar.dma_start(wih1T_t[:], d["wih1T"][:])
    whh1T_t = wk.tile([LH, 512], BF16, name="whh1T_t")
    nc.scalar.dma_start(whh1T_t[:], d["whh1T"][:])
    b0c_t = wk.tile([128, 4], F32, name="b0c_t")
    nc.sync.dma_start(b0c_t[:], d["b0c"][:])
    b1t8_t = wk.tile([128, 4, 8, BL], F32, name="b1t8_t")
    nc.sync.dma_start(b1t8_t[:], d["b1t8"][:])
    fc1T_t = wk.tile([LH, LH // 2], BF16, name="fc1T_t")
    nc.sync.dma_start(fc1T_t[:], d["fc1T"][:])
    fc1b_t = wk.tile([LH // 2, 1], F32, name="fc1b_t")
    nc.sync.dma_start(fc1b_t[:], d["fc1b"][:])
    fc2T_t = wk.tile([LH // 2, NC_OUT], BF16, name="fc2T_t")
    nc.sync.dma_start(fc2T_t[:], d["fc2T"][:])
    fc2b_t = wk.tile([NC_OUT, 1], F32, name="fc2b_t")
    nc.sync.dma_start(fc2b_t[:], d["fc2b"][:])

    # ================= GCN =================
    # Phase pools: LIFO per side; big tensors phase-scoped to fit SBUF.
    ps_cm = tc.tile_pool(name="gps", bufs=4, space="PSUM")
    ps = ps_cm.__enter__()
    pool_B = tc.alloc_tile_pool(name="pB", bufs=1, side=RIGHT)   # g1p

    # --- A-mm1: G1[m,(f,b,t)] = ah @ xa
    g1 = pool_A.tile([N, FIN, BL, T], BF16, name="g1")
    for fc in range(FIN):
        p = ps.tile([N, BL, T], F32, name="p_amm1", tag="mmp")
        nc.tensor.matmul(p[:], ah[:], xa_t[:, fc, :, :], start=True, stop=True)
        if fc % 2 == 0:
            nc.scalar.copy(g1[:, fc, :, :], p[:])
        else:
            nc.vector.tensor_copy(g1[:, fc, :, :], p[:])
    # big LSTM input-projection weights: start their HBM pulls now
    nc.scalar.dma_start(wih0_t[:], d["wih0"][:])
    nc.scalar.dma_start(wih0b_t[:], d["wih0b"][:])
    # --- thin transpose: G1 -> G1p [(jm,f), (m',b,t)]
    g1p = pool_B.tile([2 * FIN, 32, BL, T], BF16, name="g1p")
    for mp in range(32):
        # dst [(jm,f), b, t] <- src g1[{mp, 32+mp}, f, b, t]
        eng = [nc.sync, nc.scalar, nc.gpsimd][mp % 3]
        eng.dma_start(g1p[:, mp, :, :], g1[mp::32, :, :, :])
    pool_A.release()
    pool_C = tc.alloc_tile_pool(name="pC", bufs=1, side=LEFT)    # h1
    # --- W-mm1: H1[(jm,o),(m',b,t)]; copies on ACT, bn_stats inline on DVE
    # so the BN1 allreduce can fire the moment the last chunk lands.
    h1 = pool_C.tile([128, 32, BL, T], BF16, name="h1")
    st1 = wk.tile([128, 32, 6], F32, name="st1")
    for c in range(32):
        p = ps.tile([128, BL, T], F32, name="p_wmm1", tag="mmp")
        nc.tensor.matmul(p[:], w1d_t[:], g1p[:, c, :, :], start=True, stop=True)
        nc.scalar.copy(h1[:, c, :, :], p[:])
        nc.vector.bn_stats(st1[:, c, :], h1[:, c, :, :].opt())
    pool_B.release()
    pool_G = tc.alloc_tile_pool(name="pG", bufs=1, side=RIGHT)   # h2 (hs)
    h2 = pool_G.tile([128, 32, BL, T], BF16, name="h2")
    # --- BN1
    sc1, bs1 = _bn_block(nc, tc, wk, st1, g1r, b1r, dram, foldm_t, "1")
    x2 = pool_L.tile([128, 32, BL, T], BF16, name="x2")
    for c in range(32):
        if c % 2 == 0:
            nc.scalar.activation(x2[:, c, :, :], h1[:, c, :, :], AF.Relu,
                                 bias=bs1[:], scale=sc1[:])
        else:
            nc.vector.tensor_scalar(x2[:, c, :, :], h1[:, c, :, :],
                                    sc1[:, 0:1], bs1[:, 0:1],
                                    ALU.mult, ALU.add)
            nc.vector.tensor_scalar_max(x2[:, c, :, :], x2[:, c, :, :], 0.0)
    pool_C.release()
    pool_E = tc.alloc_tile_pool(name="pE", bufs=1, side=LEFT)    # z2p
    # ================= GCN layer 2 =================
    # W-mm2 per chunk -> staging -> scatter-DMA directly into transposed Z2p.
    # Z2 chunk [(jm,c2),(b,t)] scatters to Z2p [(k,n),(c2',b,t)], c2=k*32+c2'.
    z2p = pool_E.tile([128, 32, BL, T], BF16, name="z2p")
    with tc.tile_pool(name="stg2", bufs=10) as stg2p:
        for c in range(32):
            p = ps.tile([128, BL, T], F32, name="p_wmm2", tag="mmp")
            nc.tensor.matmul(p[:], w2d_t[:], x2[:, c, :, :], start=True, stop=True)
            stg = stg2p.tile([128, BL, T], BF16, name="stg")
            nc.vector.tensor_copy(stg[:], p[:])
            for jm in range(2):
                eng = [nc.sync, nc.gpsimd, nc.scalar][(2 * c + jm) % 3]
                # dst partitions {k*64+jm*32+c : k in 0,1}; iter (k,(c2',b,t))
                eng.dma_start(z2p[jm * 32 + c::64, :, :, :],
                              stg[jm * 64:(jm + 1) * 64, :, :])
    # ====== xp0 (x2-half, kg 0/1): fills PE while z2p scatter transfers land
    xps_cm = tc.tile_pool(name="xps", bufs=1, space="PSUM")
    xps = xps_cm.__enter__()
    xp_ps = {}
    for kg in (0, 1):
        pxp = xps.tile([128, BL, T], F32, name=f"p_xp{kg}")
        for c in range(32):
            nc.tensor.matmul(pxp[:], wih0_t[:, c, kg, :], x2[:, c, :, :],
                             start=(c == 0), stop=False)
        xp_ps[kg] = pxp
    # --- A-mm2: H2 [(k,m),(c2',b,t)]; BN2 runs in THIS layout (no transpose
    # back): per-chunk c2' the channel is fixed per partition-half, so ACT
    # per-partition scale/bias still works with [128,32] scale tiles.
    st2 = wk.tile([128, 32, 6], F32, name="st2")
    for c in range(32):
        p = ps.tile([128, BL, T], F32, name="p_amm2", tag="mmp")
        nc.tensor.matmul(p[:], ahd[:], z2p[:, c, :, :], start=True, stop=True)
        nc.scalar.copy(h2[:, c, :, :], p[:])
        nc.vector.bn_stats(st2[:, c, :], h2[:, c, :, :].opt())
    pool_E.release()
    # --- BN2 stats: per (partition,(k,m)), chunk c2' -> fold over m -> allreduce
    mv2 = wk.tile([128, 32, 2], F32, name="mv2")
    for c in range(32):
        nc.vector.bn_aggr(mv2[:, c, :], st2[:, c, :])
    n2 = float(BL * T)
    sums2 = wk.tile([128, 32, 2], F32, name="sums2")
    msq2 = wk.tile([128, 32], F32, name="msq2")
    nc.vector.tensor_tensor(msq2[:], mv2[:, :, 0], mv2[:, :, 0], ALU.mult)
    nc.vector.tensor_tensor(sums2[:, :, 1], mv2[:, :, 1], msq2[:], ALU.add)
    nc.scalar.mul(sums2[:, :, 0], mv2[:, :, 0], n2)
    nc.scalar.mul(sums2[:, :, 1], sums2[:, :, 1], n2)
    # fold over m-partitions within each k-half: out[p=(k,*)] = sum_m sums2[(k,m)]
    bi2 = dram.tile([128, 64], F32, name="bi2")
    bo2 = dram.tile([128, 64], F32, name="bo2")
    pf2 = ps.tile([128, 64], F32, name="pf2", tag="mmp")
    nc.tensor.matmul(pf2[:], fm2_t[:], sums2[:].opt(), start=True, stop=True)
    folded = wk.tile([128, 32, 2], F32, name="folded")
    nc.scalar.copy(folded[:], pf2[:])
    nc.sync.dma_start(bi2[:], folded[:])
    nc.gpsimd.collective_compute(
        "AllReduce", ALU.add,
        replica_groups=[list(range(NCORES))],
        ins=[bi2[:].opt()], outs=[bo2[:].opt()])
    # ====== xp0 (x2-half, kg 2/3): fills the BN2-allreduce + apply window
    for kg in (2, 3):
        pxp = xps.tile([128, BL, T], F32, name=f"p_xp{kg}")
        for c in range(32):
            nc.tensor.matmul(pxp[:], wih0_t[:, c, kg, :], x2[:, c, :, :],
                             start=(c == 0), stop=False)
        xp_ps[kg] = pxp
    red2 = wk.tile([128, 32, 2], F32, name="red2")
    nc.sync.dma_start(red2[:], bo2[:])
    tot2 = wk.tile([128, 32, 2], F32, name="tot2")
    nc.scalar.mul(tot2[:], red2[:], 1.0 / S_TOT)
    var2 = wk.tile([128, 32], F32, name="var2")
    m22 = wk.tile([128, 32], F32, name="m22")
    nc.vector.tensor_tensor(m22[:], tot2[:, :, 0], tot2[:, :, 0], ALU.mult)
    nc.vector.tensor_tensor(var2[:], tot2[:, :, 1], m22[:], ALU.subtract)
    eps2 = wk.tile([128, 32], F32, name="eps2")
    nc.vector.memset(eps2[:], EPS)
    vpe = wk.tile([128, 32], F32, name="vpe")
    nc.vector.tensor_tensor(vpe[:], var2[:], eps2[:], ALU.add)
    sd2 = wk.tile([128, 32], F32, name="sd2")
    nc.scalar.activation(sd2[:], vpe[:], AF.Sqrt)
    rs2 = wk.tile([128, 32], F32, name="rs2")
    nc.vector.reciprocal(rs2[:], sd2[:])
    scf = wk.tile([128, 32], F32, name="scf")
    nc.vector.tensor_tensor(scf[:], rs2[:], g2k[:], ALU.mult)
    mscf = wk.tile([128, 32], F32, name="mscf")
    nc.vector.tensor_tensor(mscf[:], tot2[:, :, 0], scf[:], ALU.mult)
    bsf = wk.tile([128, 32], F32, name="bsf")
    nc.vector.tensor_tensor(bsf[:], b2k[:], mscf[:], ALU.subtract)
    # --- BN2-apply + relu in place on h2 (becomes X3 in (k,m)-layout)
    for c in range(32):
        if c % 2 == 0:
            nc.scalar.activation(h2[:, c, :, :], h2[:, c, :, :], AF.Relu,
                                 bias=bsf[:, c:c + 1], scale=scf[:, c:c + 1])
        else:
            nc.vector.tensor_scalar(h2[:, c, :, :], h2[:, c, :, :],
                                    scf[:, c:c + 1], bsf[:, c:c + 1],
                                    ALU.mult, ALU.add)
            nc.vector.tensor_scalar_max(h2[:, c, :, :], h2[:, c, :, :], 0.0)
    # ====== xp0 second half: += Wih0b @ X3 + b0 (residual folded into mm) ======
    xp0 = wk.tile([128, 4, BL, T], F32, name="xp0")
    for kg in range(4):
        pxp = xp_ps[kg]
        for c in range(32):
            nc.tensor.matmul(pxp[:], wih0b_t[:, c, kg, :], h2[:, c, :, :],
                             start=False, stop=(c == 31))
        nc.scalar.activation(xp0[:, kg, :, :], pxp[:], AF.Identity,
                             bias=b0c_t[:, kg:kg + 1], scale=1.0)
    xps_cm.__exit__(None, None, None)
    if dbg is not None:
        nc.sync.dma_start(dbg[1][:], xp0[:])
    pool_G.release()
    pool_L.release()
    ps_cm.__exit__(None, None, None)

    # ================= LSTM scan =================
    # L0 unchanged (4 whh0 mm + 3 ACT + 4 DVE per slot, high-prio chain).
    # L1 revamped:
    #   - wih1 @ h0 precomputed in 8-step blocks (4 matmuls per 8 slots into
    #     a held PSUM block preloaded with b1), so slots carry only whh1's 4.
    #   - tanh-only gates: i,f,o rows of Wih1/Whh1/b1 pre-halved on host, so
    #     ONE Tanh ACT covers all 4 gates; sigma = 0.5*tanh+0.5 on DVE.
    #   - L1 lags L0 by LAG slots (needs the h0 block complete).
    LAG = 9
    sstack = ExitStack()
    sps = sstack.enter_context(tc.tile_pool(name="sps", bufs=3, space="PSUM"))
    bps = sstack.enter_context(tc.tile_pool(name="bps", bufs=2, space="PSUM"))
    sgt = sstack.enter_context(tc.tile_pool(name="sgt", bufs=3))
    suv = sstack.enter_context(tc.tile_pool(name="suv", bufs=3))
    sth = sstack.enter_context(tc.tile_pool(name="sth", bufs=3))
    shh = sstack.enter_context(tc.tile_pool(name="shh", bufs=3))
    sst = sstack.enter_context(tc.tile_pool(name="sst", bufs=1))

    S0 = sst.tile([128, 2, BL], F32, name="S0")    # [tanh(g), c] for L0
    S1 = sst.tile([128, 5, BL], F32, name="S1")    # [i,f,o,g,c] for L1
    nc.vector.memset(S0[:], 0.0)
    nc.vector.memset(S1[:], 0.0)
    hbuf0 = sst.tile([128, 2, 8, BL], BF16, name="hbuf0")  # h0 ring (2 blocks)
    gate_tok = sst.tile([128, 1], F32, name="gate_tok")
    nc.vector.memset(gate_tok[:], 1.0)
    pb_blocks = {}
    h1_prev = None
    h1_last = None
    for t in range(T + LAG):
        has0 = t < T
        s1 = t - LAG
        has1 = 0 <= s1 < T
        # --- L1 input-projection block: pb[k,s',b] = b1 + wih1 @ h0[8j+s']
        if t % 8 == 0 and 8 <= t <= T:
            j = t // 8 - 1
            pb = bps.tile([128, 4, 8, BL], F32, name="pb", tag="pb")
            nc.vector.tensor_copy(pb[:], b1t8_t[:])
            for k in range(4):
                nc.tensor.matmul(pb[:, k, :, :],
                                 wih1T_t[:, k * LH:(k + 1) * LH],
                                 hbuf0[:, j % 2, :, :],
                                 start=False, stop=True, skip_group_check=True)
            pb_blocks[j] = pb
        # --- L0 psum preload + recurrent matmuls (critical path)
        if has0:
            pt0 = sps.tile([128, 4 * BL], F32, name="pt0", tag="pt0", bufs=3)
            nc.vector.tensor_copy(pt0[:], xp0[:, :, :, t])
            if t >= 1:
                hp0 = hbuf0[:, ((t - 1) // 8) % 2, (t - 1) % 8, :]
                with tc.high_priority(offset=45):
                    # g-gate first so tanh(g) can start while i,f,o stream
                    for k in (3, 0, 1, 2):
                        nc.tensor.matmul(pt0[:, k * BL:(k + 1) * BL],
                                         whh0T_t[:, k * LH:(k + 1) * LH],
                                         hp0,
                                         start=False, stop=True,
                                         skip_group_check=True)
        # --- L1 recurrent matmuls into its block slot
        if has1 and s1 >= 1:
            pbc = pb_blocks[s1 // 8]
            for k in range(4):
                nc.tensor.matmul(pbc[:, k, s1 % 8, :],
                                 whh1T_t[:, k * LH:(k + 1) * LH],
                                 h1_prev[:],
                                 start=False, stop=True, skip_group_check=True)
        # --- L0 cell update
        if has0:
            with tc.high_priority(offset=45):
                gt0 = sgt.tile([128, 3, BL], F32)
                # tanh(g) first: its matmul lands first, so it overlaps the
                # sigmoid instead of serializing after it on ACT
                nc.scalar.activation(S0[:, 0, :], pt0[:, 3 * BL:4 * BL], AF.Tanh)
                nc.scalar.activation(gt0[:], pt0[:, 0:3 * BL], AF.Sigmoid)
                uv0 = suv.tile([128, 2, BL], F32)
                nc.vector.tensor_tensor(uv0[:], gt0[:, 0:2, :], S0[:], ALU.mult)
                nc.vector.tensor_tensor(S0[:, 1, :], uv0[:, 0, :], uv0[:, 1, :],
                                        ALU.add)
                th0 = sth.tile([128, BL], F32)
                nc.scalar.activation(th0[:], S0[:, 1, :], AF.Tanh)
                nc.vector.tensor_tensor(hbuf0[:, (t // 8) % 2, t % 8, :],
                                        gt0[:, 2, :], th0[:], ALU.mult)
                # gate_tok = th0*0 + 1 : data-dep marker ordering L1 after L0
                # (issued after hn0 so it never sits on the recurrence chain)
                nc.vector.tensor_scalar(gate_tok[:], th0[:, 0:1], 0.0, 1.0,
                                        ALU.mult, ALU.add)
        # --- L1 cell update (tanh-trick); slightly demoted so its tail ACTs
        # sort AFTER the next L0 chain ops in the static engine queues
        if has1:
            with tc.high_priority(offset=-15):
                pbc = pb_blocks[s1 // 8]
                scl = gate_tok[:, 0:1] if has0 else 1.0
                nc.scalar.activation(S1[:, 0:4, :], pbc[:, :, s1 % 8, :],
                                     AF.Tanh, scale=scl)
                nc.vector.tensor_scalar(S1[:, 0:3, :], S1[:, 0:3, :], 0.5, 0.5,
                                        ALU.mult, ALU.add)
                uv1 = suv.tile([128, 2, BL], F32, name="uv1", tag="uv1")
                nc.vector.tensor_tensor(uv1[:], S1[:, 0:2, :], S1[:, 3:5, :],
                                        ALU.mult)
                nc.vector.tensor_tensor(S1[:, 4, :], uv1[:, 0, :], uv1[:, 1, :],
                                        ALU.add)
                th1 = sth.tile([128, BL], F32, name="th1", tag="th1")
                nc.scalar.activation(th1[:], S1[:, 4, :], AF.Tanh)
                hn1 = shh.tile([128, BL], BF16)
                nc.vector.tensor_tensor(hn1[:], S1[:, 2, :], th1[:], ALU.mult)
            h1_prev = hn1
            if s1 == T - 1:
                h1_last = hn1
    if dbg is not None:
        hl32 = wk.tile([LH, BL], F32)
        nc.scalar.copy(hl32[:], h1_last[:])
        nc.sync.dma_start(dbg[2][:], hl32[:])

    # ================= FC head =================
    p1 = sps.tile([LH // 2, BL], F32, bufs=1)
    nc.tensor.matmul(p1[:], fc1T_t[:], h1_last[:], start=True, stop=True)
    z1 = wk.tile([LH // 2, BL], BF16)
    nc.scalar.activation(z1[:], p1[:], AF.Relu, bias=fc1b_t[:, 0:1], scale=1.0)
    p2 = sps.tile([NC_OUT, BL], F32, bufs=1)
    nc.tensor.matmul(p2[:], fc2T_t[:], z1[:], start=True, stop=True)
    z2o = wk.tile([NC_OUT, BL], F32)
    nc.scalar.activation(z2o[:], p2[:], AF.Identity, bias=fc2b_t[:, 0:1], scale=1.0)
    nc.sync.dma_start(y[:], z2o[:])
    sstack.close()
    stack.close()


# ---------------------------------------------------------------------------
# host packing
# ---------------------------------------------------------------------------
def _pack_inputs(x, adjacency, w1, gamma1, beta1, w2, gamma2, beta2,
                 Wih0, Whh0, bih0, bhh0, Wih1, Whh1, bih1, bhh1,
                 fc1_w, fc1_b, fc2_w, fc2_b):
    x = _f32(x)
    xa_full = np.ascontiguousarray(x.transpose(1, 3, 0, 2))  # [N, F, B, T]

    w1 = _f32(w1); w2 = _f32(w2)
    w1d = np.zeros((2 * FIN, 128), np.float32)
    w2d = np.zeros((128, 128), np.float32)
    for jm in range(2):
        w1d[jm * FIN:(jm + 1) * FIN, jm * 64:(jm + 1) * 64] = w1
        w2d[jm * 64:(jm + 1) * 64, jm * 64:(jm + 1) * 64] = w2

    q = np.arange(128)
    foldm = (q[:, None] % 64 == q[None, :] % 64).astype(np.float32)

    g1rr = np.tile(_f32(gamma1), 2).reshape(128, 1)
    b1rr = np.tile(_f32(beta1), 2).reshape(128, 1)

    W0p = _f32(Wih0)[GPERM]                       # [512, 4096]
    W0r = W0p.reshape(512, 2, 32, 64)             # [g, jm, m', o]
    wih0 = np.ascontiguousarray(
        W0r.transpose(1, 3, 2, 0).reshape(128, 32, 4, 128))
    W0n = W0p.reshape(512, 64, 2, 32)             # [g, n, k, c2']
    wih0b = np.ascontiguousarray(
        W0n.transpose(2, 1, 3, 0).reshape(128, 32, 4, 128))
    fm2 = np.zeros((128, 128), np.float32)
    fm2[:64, :64] = 1.0
    fm2[64:, 64:] = 1.0
    g2 = _f32(gamma2); be2 = _f32(beta2)
    g2kk = np.concatenate([np.tile(g2[:32], (64, 1)), np.tile(g2[32:], (64, 1))])
    b2kk = np.concatenate([np.tile(be2[:32], (64, 1)), np.tile(be2[32:], (64, 1))])
    whh0T = np.ascontiguousarray(_f32(Whh0)[GPERM].T)   # [128, 512]
    # L1 tanh-trick: sigma(x) = 0.5*tanh(x/2)+0.5 -> halve i,f,o rows (0:384
    # in permuted i,f,o,g order); g rows (384:512) stay full for tanh.
    halv = np.concatenate([np.full(384, 0.5, np.float32),
                           np.ones(128, np.float32)])
    wih1T = np.ascontiguousarray((_f32(Wih1)[GPERM] * halv[:, None]).T)
    whh1T = np.ascontiguousarray((_f32(Whh1)[GPERM] * halv[:, None]).T)
    b0 = (_f32(bih0) + _f32(bhh0))[GPERM]
    b0c = np.ascontiguousarray(b0.reshape(4, 128).T)    # [128, 4]
    b1 = (_f32(bih1) + _f32(bhh1))[GPERM] * halv
    b1c = b1.reshape(4, 128).T                          # [128, 4]
    b1t8 = np.ascontiguousarray(np.broadcast_to(
        b1c[:, :, None, None], (128, 4, 8, BL)).copy())

    common = {
        "adj": _f32(adjacency), "eye": np.eye(N, dtype=np.float32),
        "w1d": _bf16(w1d), "w2d": _bf16(w2d), "foldm": foldm,
        "g1r": g1rr, "b1r": b1rr, "g2k": np.ascontiguousarray(g2kk),
        "b2k": np.ascontiguousarray(b2kk), "fm2": fm2,
        "wih0": _bf16(wih0), "wih0b": _bf16(wih0b), "whh0T": _bf16(whh0T),
        "wih1T": _bf16(wih1T), "whh1T": _bf16(whh1T),
        "b0c": b0c, "b1t8": b1t8,
        "fc1T": _bf16(_f32(fc1_w).T), "fc1b": _f32(fc1_b).reshape(-1, 1),
        "fc2T": _bf16(_f32(fc2_w).T), "fc2b": _f32(fc2_b).reshape(-1, 1),
    }
    in_maps = []
    for c in range(NCORES):
        m = dict(common)
        m["xa"] = _bf16(xa_full[:, :, c * BL:(c + 1) * BL, :])
        in_maps.append(m)
    return in_maps


_CACHE = {}


def kernel(**inputs):
    global LAST_EXEC_NS
    debug = bool(int(os.environ.get("STGCN_DEBUG", "0")))
    key = ("dbg" if debug else "std")
    if key not in _CACHE:
        _CACHE[key] = _build(debug=debug)
    nc = _CACHE[key]
    in_maps = _pack_inputs(**inputs)
    kw = {}
    tdir = os.environ.get("STGCN_TRACE_DIR")
    if tdir:
        kw["tmpdir"] = tdir
    res = run_bass_kernel_spmd(nc, in_maps, core_ids=list(range(NCORES)), **kw)
    LAST_EXEC_NS = res.exec_time_ns
    if debug:
        kernel.debug_results = res.results
    out = np.zeros((B, NC_OUT), np.float32)
    for c in range(NCORES):
        out[c * BL:(c + 1) * BL, :] = np.asarray(res.results[c]["y"], np.float32).T
    return out



# revision 38
# speedup vs baseline: 1.0472x; 1.0472x over previous
"""STGCN fully on-device for 8 Trainium2 NeuronCores.

Data-parallel over batch (4 examples/core). Entire forward runs in ONE Bass
SPMD launch per call:
  - adjacency normalization on device
  - GCN1: A-first matmul (thin), transpose, W-matmul (block-diag packed,
    128-partition), BN1 stats via bn_stats + cross-core AllReduce, fused
    BN-apply+ReLU on ACT
  - GCN2: W-first matmul, DMA transpose, A-matmul (block-diag), DMA
    transpose back, BN2 (same path), residual add
  - LSTM0+LSTM1 pipelined scan: 129 slots, PSUM-preloaded xp/bias, 4+8
    matmuls/slot, fused sigmoid/tanh across both layers
  - FC head, output DMA

Layout glossary (per core, BL=4):
  jm in {0,1} node-half, node n = jm*32+m' ; channel o in [0,64)
  "L-layout":  [p=(jm,o)=128, (m'=32, b=4, t=128)]
  gates order: i, f, o, g  (PyTorch i,f,g,o permuted so sigmoid gates are
  adjacent)
"""
import os

import numpy as np
import ml_dtypes

import concourse.bass as bass
import concourse.tile as tile
from concourse import mybir
from concourse.bass_utils import run_bass_kernel_spmd

F32 = mybir.dt.float32
BF16 = mybir.dt.bfloat16
AF = mybir.ActivationFunctionType
ALU = mybir.AluOpType

B, N, T, FIN = 32, 64, 128, 9
GH, LH, NC_OUT = 64, 128, 16
NCORES = 8
BL = B // NCORES          # 4 examples per core
EPS = 1e-5
S_TOT = float(B * N * T)  # BN sample count (full batch)

# gate permutation: torch order i,f,g,o -> i,f,o,g
GPERM = np.concatenate([np.arange(0, 128), np.arange(128, 256),
                        np.arange(384, 512), np.arange(256, 384)])

LAST_EXEC_NS = None


def _bf16(a):
    return np.ascontiguousarray(np.asarray(a, np.float32).astype(ml_dtypes.bfloat16))


def _f32(a):
    return np.ascontiguousarray(np.asarray(a, np.float32))


# ---------------------------------------------------------------------------
# walrus wait-cap workaround (same as baseline)
# ---------------------------------------------------------------------------
def _split_excess_waits(nc):
    fix_id = 0
    for fn in nc.m.functions:
        for blk in fn.blocks:
            out = []
            changed = False
            for inst in blk.instructions:
                si = inst.sync_info
                waits = list(si.on_wait) if si and si.on_wait else []
                cap = 2 if isinstance(inst, mybir.InstEventSemaphore) else 1
                if len(waits) > cap:
                    extra, keep = waits[: len(waits) - cap], waits[len(waits) - cap:]
                    for w in extra:
                        nop = mybir.InstNoOp(name=f"waitfix-{fix_id}")
                        fix_id += 1
                        nop.engine = inst.engine
                        nop.sync_info = mybir.SyncInfo(on_wait=[w], on_update=[])
                        nop.debug = inst.debug
                        nc.register_instruction(nop, overwrite=True)
                        out.append(nop)
                    si.on_wait = keep
                    changed = True
                out.append(inst)
            if changed:
                blk.instructions = out
    return nc


# ---------------------------------------------------------------------------
# device program
# ---------------------------------------------------------------------------
def _build(debug=False):
    nc = bass.Bass(num_devices=NCORES)
    d = {}

    def din(name, shape, dt):
        d[name] = nc.dram_tensor(name, shape, dt, kind="ExternalInput")
        return d[name]

    xa = din("xa", [N, FIN, BL, T], BF16)          # x for A-mm1: [n,(f,b,t)]
    adj = din("adj", [N, N], F32)
    eye = din("eye", [N, N], F32)
    w1d = din("w1d", [2 * FIN, 128], BF16)         # blockdiag(w1,w1)
    w2d = din("w2d", [128, 128], BF16)             # blockdiag over jm of w2
    foldm = din("foldm", [128, 128], F32)          # mod-64 partition fold
    g1r = din("g1r", [128, 1], F32)
    b1r = din("b1r", [128, 1], F32)
    g2k = din("g2k", [128, 32], F32)               # gamma2 in (k,c2') layout
    b2k = din("b2k", [128, 32], F32)
    wih0 = din("wih0", [128, 32, 4, 128], BF16)    # [r=(jm,o), kchunk=m', kg, grow]
    wih0b = din("wih0b", [128, 32, 4, 128], BF16)  # [r=(k,m), kchunk=c2', kg, grow]
    fm2 = din("fm2", [128, 128], F32)              # blockdiag(ones64) fold
    whh0T = din("whh0T", [LH, 4 * LH], BF16)
    wih1T = din("wih1T", [LH, 4 * LH], BF16)
    whh1T = din("whh1T", [LH, 4 * LH], BF16)
    b0c = din("b0c", [128, 4], F32)                # bias0 per gate-chunk col
    b1t8 = din("b1t8", [128, 4, 8, BL], F32)       # L1 block-psum preload
    fc1T = din("fc1T", [LH, LH // 2], BF16)
    fc1b = din("fc1b", [LH // 2, 1], F32)
    fc2T = din("fc2T", [LH // 2, NC_OUT], BF16)
    fc2b = din("fc2b", [NC_OUT, 1], F32)
    y = nc.dram_tensor("y", [NC_OUT, BL], F32, kind="ExternalOutput")
    if debug:
        dbg_xp0 = nc.dram_tensor("dbg_xp0", [128, 4, BL, T], F32, kind="ExternalOutput")
        dbg_h0 = nc.dram_tensor("dbg_h0", [LH, BL], F32, kind="ExternalOutput")

    with tile.TileContext(nc) as tc:
        _prog(nc, tc, d, y,
              dbg=(None, dbg_xp0, dbg_h0) if debug else None)
    _split_excess_waits(nc)
    return nc


def _bn_block(nc, tc, wk, st, gr, br, dram, foldm_t, tag):
    """Aggregate precomputed bn_stats `st` [128,32,6] -> cross-core allreduce
    -> per-partition scale/bias [128,1] f32. Returns (sc, bs)."""
    mv = wk.tile([128, 2], F32)
    nc.vector.bn_aggr(mv[:], st[:])
    # local sums: n_loc = 32*BL*T per partition
    n_loc = float(32 * BL * T)
    sums = wk.tile([128, 2], F32)
    # sums[:,0] = mean*n_loc ; sums[:,1] = (var + mean^2)*n_loc
    msq = wk.tile([128, 1], F32)
    nc.vector.tensor_tensor(msq[:], mv[:, 0:1], mv[:, 0:1], ALU.mult)
    nc.vector.tensor_tensor(sums[:, 1:2], mv[:, 1:2], msq[:], ALU.add)
    nc.scalar.mul(sums[:, 0:1], mv[:, 0:1], n_loc)
    nc.scalar.mul(sums[:, 1:2], sums[:, 1:2], n_loc)
    # allreduce over 8 cores
    bi = dram.tile([128, 2], F32)
    bo = dram.tile([128, 2], F32)
    nc.sync.dma_start(bi[:], sums[:])
    nc.gpsimd.collective_compute(
        "AllReduce", ALU.add,
        replica_groups=[list(range(NCORES))],
        ins=[bi[:].opt()], outs=[bo[:].opt()])
    red = wk.tile([128, 2], F32)
    nc.sync.dma_start(red[:], bo[:])
    # fold jm-halves (mod-64) via PE: out[p,:] = sum_q foldm[q,p]*red[q,:]
    with tc.tile_pool(name=f"bnps{tag}", bufs=1, space="PSUM") as ps:
        pf = ps.tile([128, 2], F32)
        nc.tensor.matmul(pf[:], foldm_t[:], red[:], start=True, stop=True)
        tot = wk.tile([128, 2], F32)
        nc.scalar.mul(tot[:], pf[:], 1.0 / S_TOT)   # [mean, E[x^2]]
    var = wk.tile([128, 1], F32)
    m2 = wk.tile([128, 1], F32)
    nc.vector.tensor_tensor(m2[:], tot[:, 0:1], tot[:, 0:1], ALU.mult)
    nc.vector.tensor_tensor(var[:], tot[:, 1:2], m2[:], ALU.subtract)
    epst = wk.tile([128, 1], F32)
    nc.vector.memset(epst[:], EPS)
    sd = wk.tile([128, 1], F32)
    nc.scalar.activation(sd[:], var[:], AF.Sqrt, bias=epst[:, 0:1], scale=1.0)
    rs = wk.tile([128, 1], F32)
    nc.vector.reciprocal(rs[:], sd[:])
    sc = wk.tile([128, 1], F32)
    nc.vector.tensor_tensor(sc[:], rs[:], gr[:], ALU.mult)
    mscale = wk.tile([128, 1], F32)
    nc.vector.tensor_tensor(mscale[:], tot[:, 0:1], sc[:], ALU.mult)
    bs = wk.tile([128, 1], F32)
    nc.vector.tensor_tensor(bs[:], br[:], mscale[:], ALU.subtract)
    return sc, bs


def _prog(nc, tc, d, y, dbg=None):
    from contextlib import ExitStack
    stack = ExitStack()
    wk = stack.enter_context(tc.tile_pool(name="wk", bufs=1))       # persistent small
    dram = stack.enter_context(tc.tile_pool(name="drb", bufs=1, space="DRAM"))

    # ---------------- warm up the collective fabric (absorbs comm init +
    # launch skew off the BN1 allreduce's critical path)
    warm = wk.tile([128, 1], F32, name="warm")
    nc.vector.memset(warm[:], 0.0)
    wi = dram.tile([128, 1], F32, name="wi")
    wo = dram.tile([128, 1], F32, name="wo")
    nc.gpsimd.dma_start(wi[:], warm[:])
    nc.gpsimd.collective_compute(
        "AllReduce", ALU.add,
        replica_groups=[list(range(NCORES))],
        ins=[wi[:].opt()], outs=[wo[:].opt()])

    # ---------------- critical-path inputs first: xa + adjacency + w1d
    LEFT, RIGHT = "left", "right"
    pool_L = tc.alloc_tile_pool(name="pL", bufs=1, side=LEFT)    # x2 (long)
    pool_A = tc.alloc_tile_pool(name="pA", bufs=1, side=LEFT)    # xa, g1
    adj_t = wk.tile([N, N], F32, name="adj_t")
    eye_t = wk.tile([N, N], F32, name="eye_t")
    xa_t = pool_A.tile([N, FIN, BL, T], BF16, name="xa_t")
    w1d_t = wk.tile([2 * FIN, 128], BF16, name="w1d_t")
    with tc.high_priority():
        nc.sync.dma_start(xa_t[:, 0:3, :, :], d["xa"][:, 0:3, :, :])
        nc.scalar.dma_start(xa_t[:, 3:6, :, :], d["xa"][:, 3:6, :, :])
        nc.gpsimd.dma_start(xa_t[:, 6:9, :, :], d["xa"][:, 6:9, :, :])
        nc.sync.dma_start(adj_t[:], d["adj"][:])
        nc.sync.dma_start(eye_t[:], d["eye"][:])
        nc.sync.dma_start(w1d_t[:], d["w1d"][:])

    # ---------------- PE p-state warmup (reach 2.4GHz before GCN matmuls)
    wz = wk.tile([128, 512], BF16, name="wz")
    nc.vector.memset(wz[:], 0.0)
    with tc.tile_pool(name="wups", bufs=1, space="PSUM") as wps:
        wp = wps.tile([128, 512], F32, name="wp")
        for _ in range(8):
            nc.tensor.matmul(wp[:], wz[:, 0:128], wz[:], start=True, stop=True)

    # ---------------- remaining small constants (gpsimd queue: keeps the
    # sync ring free for xa/adj so A-mm1 can start early)
    foldm_t = wk.tile([128, 128], F32, name="foldm_t")
    nc.gpsimd.dma_start(foldm_t[:], d["foldm"][:])
    g1r = wk.tile([128, 1], F32, name="g1r")
    nc.gpsimd.dma_start(g1r[:], d["g1r"][:])
    b1r = wk.tile([128, 1], F32, name="b1r")
    nc.gpsimd.dma_start(b1r[:], d["b1r"][:])
    g2k = wk.tile([128, 32], F32, name="g2k")
    nc.gpsimd.dma_start(g2k[:], d["g2k"][:])
    b2k = wk.tile([128, 32], F32, name="b2k")
    nc.gpsimd.dma_start(b2k[:], d["b2k"][:])
    fm2_t = wk.tile([128, 128], F32, name="fm2_t")
    nc.gpsimd.dma_start(fm2_t[:], d["fm2"][:])
    w2d_t = wk.tile([128, 128], BF16, name="w2d_t")
    nc.gpsimd.dma_start(w2d_t[:], d["w2d"][:])

    a1 = wk.tile([N, N], F32)
    nc.vector.tensor_tensor(a1[:], adj_t[:], eye_t[:], ALU.add)
    deg = wk.tile([N, 1], F32)
    nc.vector.tensor_reduce(deg[:], a1[:], mybir.AxisListType.X, ALU.add)
    sdg = wk.tile([N, 1], F32)
    nc.scalar.activation(sdg[:], deg[:], AF.Sqrt)
    dinv = wk.tile([N, 1], F32)
    nc.vector.reciprocal(dinv[:], sdg[:])
    a2 = wk.tile([N, N], F32)
    nc.vector.tensor_scalar(a2[:], a1[:], dinv[:], None, ALU.mult)
    ones1 = wk.tile([1, N], F32)
    nc.vector.memset(ones1[:], 1.0)
    with tc.tile_pool(name="adjps", bufs=1, space="PSUM") as ps:
        pdt = ps.tile([1, N], F32)
        nc.tensor.transpose(pdt[:], dinv[:], eye_t[:])
        dT = wk.tile([1, N], F32)
        nc.scalar.copy(dT[:], pdt[:])
        pbc = ps.tile([N, N], F32)
        nc.tensor.matmul(pbc[:], ones1[:], dT[:], start=True, stop=True)
        ah = wk.tile([N, N], BF16)       # normalized adjacency, bf16
        nc.vector.tensor_tensor(ah[:], a2[:], pbc[:], ALU.mult)
    ahd = wk.tile([128, 128], BF16)      # blockdiag(ah, ah)
    nc.vector.memset(ahd[:], 0.0)
    nc.scalar.copy(ahd[0:64, 0:64], ah[:])
    nc.scalar.copy(ahd[64:128, 64:128], ah[:])

    # ---------------- LSTM weights; the two big 4MB tiles (wih0/wih0b) are
    # triggered AFTER A-mm1 below so their HBM traffic cannot delay xa/adj.
    wih0_t = wk.tile([128, 32, 4, 128], BF16, name="wih0_t")
    wih0b_t = wk.tile([128, 32, 4, 128], BF16, name="wih0b_t")
    whh0T_t = wk.tile([LH, 512], BF16, name="whh0T_t")
    nc.scalar.dma_start(whh0T_t[:], d["whh0T"][:])
    wih1T_t = wk.tile([LH, 512], BF16, name="wih1T_t")
    nc.scalar.dma_start(wih1T_t[:], d["wih1T"][:])
    whh1T_t = wk.tile([LH, 512], BF16, name="whh1T_t")
    nc.scalar.dma_start(whh1T_t[:], d["whh1T"][:])
    b0c_t = wk.tile([128, 4], F32, name="b0c_t")
    nc.sync.dma_start(b0c_t[:], d["b0c"][:])
    b1t8_t = wk.tile([128, 4, 8, BL], F32, name="b1t8_t")
    nc.sync.dma_start(b1t8_t[:], d["b1t8"][:])
    fc1T_t = wk.tile([LH, LH // 2], BF16, name="fc1T_t")
    nc.sync.dma_start(fc1T_t[:], d["fc1T"][:])
    fc1b_t = wk.tile([LH // 2, 1], F32, name="fc1b_t")
    nc.sync.dma_start(fc1b_t[:], d["fc1b"][:])
    fc2T_t = wk.tile([LH // 2, NC_OUT], BF16, name="fc2T_t")
    nc.sync.dma_start(fc2T_t[:], d["fc2T"][:])
    fc2b_t = wk.tile([NC_OUT, 1], F32, name="fc2b_t")
    nc.sync.dma_start(fc2b_t[:], d["fc2b"][:])

    # ================= GCN =================
    # Phase pools: LIFO per side; big tensors phase-scoped to fit SBUF.
    ps_cm = tc.tile_pool(name="gps", bufs=4, space="PSUM")
    ps = ps_cm.__enter__()
    pool_B = tc.alloc_tile_pool(name="pB", bufs=1, side=RIGHT)   # g1p

    # --- A-mm1: G1[m,(f,b,t)] = ah @ xa
    g1 = pool_A.tile([N, FIN, BL, T], BF16, name="g1")
    for fc in range(FIN):
        p = ps.tile([N, BL, T], F32, name="p_amm1", tag="mmp")
        nc.tensor.matmul(p[:], ah[:], xa_t[:, fc, :, :], start=True, stop=True)
        if fc % 2 == 0:
            nc.scalar.copy(g1[:, fc, :, :], p[:])
        else:
            nc.vector.tensor_copy(g1[:, fc, :, :], p[:])
    # big LSTM input-projection weights: start their HBM pulls now
    nc.scalar.dma_start(wih0_t[:], d["wih0"][:])
    nc.scalar.dma_start(wih0b_t[:], d["wih0b"][:])
    # --- thin transpose: G1 -> G1p [(jm,f), (m',b,t)]
    g1p = pool_B.tile([2 * FIN, 32, BL, T], BF16, name="g1p")
    for mp in range(32):
        # dst [(jm,f), b, t] <- src g1[{mp, 32+mp}, f, b, t]
        eng = [nc.sync, nc.scalar, nc.gpsimd][mp % 3]
        eng.dma_start(g1p[:, mp, :, :], g1[mp::32, :, :, :])
    pool_A.release()
    pool_C = tc.alloc_tile_pool(name="pC", bufs=1, side=LEFT)    # h1
    # --- W-mm1: H1[(jm,o),(m',b,t)]; copies on ACT, bn_stats inline on DVE
    # so the BN1 allreduce can fire the moment the last chunk lands.
    h1 = pool_C.tile([128, 32, BL, T], BF16, name="h1")
    st1 = wk.tile([128, 32, 6], F32, name="st1")
    for c in range(32):
        p = ps.tile([128, BL, T], F32, name="p_wmm1", tag="mmp")
        nc.tensor.matmul(p[:], w1d_t[:], g1p[:, c, :, :], start=True, stop=True)
        nc.scalar.copy(h1[:, c, :, :], p[:])
        nc.vector.bn_stats(st1[:, c, :], h1[:, c, :, :].opt())
    pool_B.release()
    pool_G = tc.alloc_tile_pool(name="pG", bufs=1, side=RIGHT)   # h2 (hs)
    h2 = pool_G.tile([128, 32, BL, T], BF16, name="h2")
    # --- BN1
    sc1, bs1 = _bn_block(nc, tc, wk, st1, g1r, b1r, dram, foldm_t, "1")
    x2 = pool_L.tile([128, 32, BL, T], BF16, name="x2")
    for c in range(32):
        if c % 2 == 0:
            nc.scalar.activation(x2[:, c, :, :], h1[:, c, :, :], AF.Relu,
                                 bias=bs1[:], scale=sc1[:])
        else:
            nc.vector.tensor_scalar(x2[:, c, :, :], h1[:, c, :, :],
                                    sc1[:, 0:1], bs1[:, 0:1],
                                    ALU.mult, ALU.add)
            nc.vector.tensor_scalar_max(x2[:, c, :, :], x2[:, c, :, :], 0.0)
    pool_C.release()
    pool_E = tc.alloc_tile_pool(name="pE", bufs=1, side=LEFT)    # z2p
    # ================= GCN layer 2 =================
    # W-mm2 per chunk -> staging -> scatter-DMA directly into transposed Z2p.
    # Z2 chunk [(jm,c2),(b,t)] scatters to Z2p [(k,n),(c2',b,t)], c2=k*32+c2'.
    z2p = pool_E.tile([128, 32, BL, T], BF16, name="z2p")
    with tc.tile_pool(name="stg2", bufs=10) as stg2p:
        for c in range(32):
            p = ps.tile([128, BL, T], F32, name="p_wmm2", tag="mmp")
            nc.tensor.matmul(p[:], w2d_t[:], x2[:, c, :, :], start=True, stop=True)
            stg = stg2p.tile([128, BL, T], BF16, name="stg")
            nc.vector.tensor_copy(stg[:], p[:])
            for jm in range(2):
                eng = [nc.sync, nc.gpsimd, nc.scalar][(2 * c + jm) % 3]
                # dst partitions {k*64+jm*32+c : k in 0,1}; iter (k,(c2',b,t))
                eng.dma_start(z2p[jm * 32 + c::64, :, :, :],
                              stg[jm * 64:(jm + 1) * 64, :, :])
    # ====== xp0 (x2-half, kg 0/1): fills PE while z2p scatter transfers land
    xps_cm = tc.tile_pool(name="xps", bufs=1, space="PSUM")
    xps = xps_cm.__enter__()
    xp_ps = {}
    for kg in (0, 1):
        pxp = xps.tile([128, BL, T], F32, name=f"p_xp{kg}")
        for c in range(32):
            nc.tensor.matmul(pxp[:], wih0_t[:, c, kg, :], x2[:, c, :, :],
                             start=(c == 0), stop=False)
        xp_ps[kg] = pxp
    # --- A-mm2: H2 [(k,m),(c2',b,t)]; BN2 runs in THIS layout (no transpose
    # back): per-chunk c2' the channel is fixed per partition-half, so ACT
    # per-partition scale/bias still works with [128,32] scale tiles.
    st2 = wk.tile([128, 32, 6], F32, name="st2")
    for c in range(32):
        p = ps.tile([128, BL, T], F32, name="p_amm2", tag="mmp")
        nc.tensor.matmul(p[:], ahd[:], z2p[:, c, :, :], start=True, stop=True)
        nc.scalar.copy(h2[:, c, :, :], p[:])
        nc.vector.bn_stats(st2[:, c, :], h2[:, c, :, :].opt())
    pool_E.release()
    # --- BN2 stats: per (partition,(k,m)), chunk c2' -> fold over m -> allreduce
    mv2 = wk.tile([128, 32, 2], F32, name="mv2")
    for c in range(32):
        nc.vector.bn_aggr(mv2[:, c, :], st2[:, c, :])
    n2 = float(BL * T)
    sums2 = wk.tile([128, 32, 2], F32, name="sums2")
    msq2 = wk.tile([128, 32], F32, name="msq2")
    nc.vector.tensor_tensor(msq2[:], mv2[:, :, 0], mv2[:, :, 0], ALU.mult)
    nc.vector.tensor_tensor(sums2[:, :, 1], mv2[:, :, 1], msq2[:], ALU.add)
    nc.scalar.mul(sums2[:, :, 0], mv2[:, :, 0], n2)
    nc.scalar.mul(sums2[:, :, 1], sums2[:, :, 1], n2)
    # fold over m-partitions within each k-half: out[p=(k,*)] = sum_m sums2[(k,m)]
    bi2 = dram.tile([128, 64], F32, name="bi2")
    bo2 = dram.tile([128, 64], F32, name="bo2")
    pf2 = ps.tile([128, 64], F32, name="pf2", tag="mmp")
    nc.tensor.matmul(pf2[:], fm2_t[:], sums2[:].opt(), start=True, stop=True)
    folded = wk.tile([128, 32, 2], F32, name="folded")
    nc.scalar.copy(folded[:], pf2[:])
    nc.sync.dma_start(bi2[:], folded[:])
    nc.gpsimd.collective_compute(
        "AllReduce", ALU.add,
        replica_groups=[list(range(NCORES))],
        ins=[bi2[:].opt()], outs=[bo2[:].opt()])
    # ====== xp0 (x2-half, kg 2/3): fills the BN2-allreduce + apply window
    for kg in (2, 3):
        pxp = xps.tile([128, BL, T], F32, name=f"p_xp{kg}")
        for c in range(32):
            nc.tensor.matmul(pxp[:], wih0_t[:, c, kg, :], x2[:, c, :, :],
                             start=(c == 0), stop=False)
        xp_ps[kg] = pxp
    red2 = wk.tile([128, 32, 2], F32, name="red2")
    nc.sync.dma_start(red2[:], bo2[:])
    tot2 = wk.tile([128, 32, 2], F32, name="tot2")
    nc.scalar.mul(tot2[:], red2[:], 1.0 / S_TOT)
    var2 = wk.tile([128, 32], F32, name="var2")
    m22 = wk.tile([128, 32], F32, name="m22")
    nc.vector.tensor_tensor(m22[:], tot2[:, :, 0], tot2[:, :, 0], ALU.mult)
    nc.vector.tensor_tensor(var2[:], tot2[:, :, 1], m22[:], ALU.subtract)
    eps2 = wk.tile([128, 32], F32, name="eps2")
    nc.vector.memset(eps2[:], EPS)
    vpe = wk.tile([128, 32], F32, name="vpe")
    nc.vector.tensor_tensor(vpe[:], var2[:], eps2[:], ALU.add)
    sd2 = wk.tile([128, 32], F32, name="sd2")
    nc.scalar.activation(sd2[:], vpe[:], AF.Sqrt)
    rs2 = wk.tile([128, 32], F32, name="rs2")
    nc.vector.reciprocal(rs2[:], sd2[:])
    scf = wk.tile([128, 32], F32, name="scf")
    nc.vector.tensor_tensor(scf[:], rs2[:], g2k[:], ALU.mult)
    mscf = wk.tile([128, 32], F32, name="mscf")
    nc.vector.tensor_tensor(mscf[:], tot2[:, :, 0], scf[:], ALU.mult)
    bsf = wk.tile([128, 32], F32, name="bsf")
    nc.vector.tensor_tensor(bsf[:], b2k[:], mscf[:], ALU.subtract)
    # --- BN2-apply + relu in place on h2 (becomes X3 in (k,m)-layout)
    for c in range(32):
        if c % 2 == 0:
            nc.scalar.activation(h2[:, c, :, :], h2[:, c, :, :], AF.Relu,
                                 bias=bsf[:, c:c + 1], scale=scf[:, c:c + 1])
        else:
            nc.vector.tensor_scalar(h2[:, c, :, :], h2[:, c, :, :],
                                    scf[:, c:c + 1], bsf[:, c:c + 1],
                                    ALU.mult, ALU.add)
            nc.vector.tensor_scalar_max(h2[:, c, :, :], h2[:, c, :, :], 0.0)
    # ====== xp0 second half: += Wih0b @ X3 + b0 (residual folded into mm) ======
    xp0 = wk.tile([128, 4, BL, T], F32, name="xp0")
    for kg in range(4):
        pxp = xp_ps[kg]
        for c in range(32):
            nc.tensor.matmul(pxp[:], wih0b_t[:, c, kg, :], h2[:, c, :, :],
                             start=False, stop=(c == 31))
        nc.scalar.activation(xp0[:, kg, :, :], pxp[:], AF.Identity,
                             bias=b0c_t[:, kg:kg + 1], scale=1.0)
    xps_cm.__exit__(None, None, None)
    if dbg is not None:
        nc.sync.dma_start(dbg[1][:], xp0[:])
    pool_G.release()
    pool_L.release()
    ps_cm.__exit__(None, None, None)

    # ================= LSTM scan =================
    # L0 unchanged (4 whh0 mm + 3 ACT + 4 DVE per slot, high-prio chain).
    # L1 revamped:
    #   - wih1 @ h0 precomputed in 8-step blocks (4 matmuls per 8 slots into
    #     a held PSUM block preloaded with b1), so slots carry only whh1's 4.
    #   - tanh-only gates: i,f,o rows of Wih1/Whh1/b1 pre-halved on host, so
    #     ONE Tanh ACT covers all 4 gates; sigma = 0.5*tanh+0.5 on DVE.
    #   - L1 lags L0 by LAG slots (needs the h0 block complete).
    LAG = 9
    sstack = ExitStack()
    sps = sstack.enter_context(tc.tile_pool(name="sps", bufs=3, space="PSUM"))
    bps = sstack.enter_context(tc.tile_pool(name="bps", bufs=2, space="PSUM"))
    sgt = sstack.enter_context(tc.tile_pool(name="sgt", bufs=3))
    suv = sstack.enter_context(tc.tile_pool(name="suv", bufs=3))
    sth = sstack.enter_context(tc.tile_pool(name="sth", bufs=3))
    shh = sstack.enter_context(tc.tile_pool(name="shh", bufs=3))
    sst = sstack.enter_context(tc.tile_pool(name="sst", bufs=1))

    S0 = sst.tile([128, 2, BL], F32, name="S0")    # [tanh(g), c] for L0
    S1 = sst.tile([128, 5, BL], F32, name="S1")    # [i,f,o,g,c] for L1
    nc.vector.memset(S0[:], 0.0)
    nc.vector.memset(S1[:], 0.0)
    hbuf0 = sst.tile([128, 2, 8, BL], BF16, name="hbuf0")  # h0 ring (2 blocks)
    gate_tok = sst.tile([128, 1], F32, name="gate_tok")
    nc.vector.memset(gate_tok[:], 1.0)
    pb_blocks = {}
    h1_prev = None
    h1_last = None
    for t in range(T + LAG):
        has0 = t < T
        s1 = t - LAG
        has1 = 0 <= s1 < T
        # --- L1 input-projection block: pb[k,s',b] = b1 + wih1 @ h0[8j+s']
        if t % 8 == 0 and 8 <= t <= T:
            j = t // 8 - 1
            pb = bps.tile([128, 4, 8, BL], F32, name="pb", tag="pb")
            nc.vector.tensor_copy(pb[:], b1t8_t[:])
            for k in range(4):
                nc.tensor.matmul(pb[:, k, :, :],
                                 wih1T_t[:, k * LH:(k + 1) * LH],
                                 hbuf0[:, j % 2, :, :],
                                 start=False, stop=True, skip_group_check=True)
            pb_blocks[j] = pb
        # --- L0 psum preload + recurrent matmuls (critical path)
        if has0:
            pt0 = sps.tile([128, 4 * BL], F32, name="pt0", tag="pt0", bufs=3)
            nc.vector.tensor_copy(pt0[:], xp0[:, :, :, t])
            if t >= 1:
                hp0 = hbuf0[:, ((t - 1) // 8) % 2, (t - 1) % 8, :]
                with tc.high_priority(offset=30):
                    # g-gate first so tanh(g) can start while i,f,o stream
                    for k in (3, 0, 1, 2):
                        nc.tensor.matmul(pt0[:, k * BL:(k + 1) * BL],
                                         whh0T_t[:, k * LH:(k + 1) * LH],
                                         hp0,
                                         start=False, stop=True,
                                         skip_group_check=True)
        # --- L1 recurrent matmuls into its block slot
        if has1 and s1 >= 1:
            pbc = pb_blocks[s1 // 8]
            for k in range(4):
                nc.tensor.matmul(pbc[:, k, s1 % 8, :],
                                 whh1T_t[:, k * LH:(k + 1) * LH],
                                 h1_prev[:],
                                 start=False, stop=True, skip_group_check=True)
        # --- L0 cell update
        if has0:
            with tc.high_priority(offset=30):
                gt0 = sgt.tile([128, 3, BL], F32)
                # tanh(g) first: its matmul lands first, so it overlaps the
                # sigmoid instead of serializing after it on ACT
                nc.scalar.activation(S0[:, 0, :], pt0[:, 3 * BL:4 * BL], AF.Tanh)
                nc.scalar.activation(gt0[:], pt0[:, 0:3 * BL], AF.Sigmoid)
                uv0 = suv.tile([128, 2, BL], F32)
                nc.vector.tensor_tensor(uv0[:], gt0[:, 0:2, :], S0[:], ALU.mult)
                nc.vector.tensor_tensor(S0[:, 1, :], uv0[:, 0, :], uv0[:, 1, :],
                                        ALU.add)
                th0 = sth.tile([128, BL], F32)
                nc.scalar.activation(th0[:], S0[:, 1, :], AF.Tanh)
                nc.vector.tensor_tensor(hbuf0[:, (t // 8) % 2, t % 8, :],
                                        gt0[:, 2, :], th0[:], ALU.mult)
                # gate_tok = th0*0 + 1 : data-dep marker ordering L1 after L0
                # (issued after hn0 so it never sits on the recurrence chain)
                nc.vector.tensor_scalar(gate_tok[:], th0[:, 0:1], 0.0, 1.0,
                                        ALU.mult, ALU.add)
        # --- L1 cell update (tanh-trick)
        if has1:
            pbc = pb_blocks[s1 // 8]
            scl = gate_tok[:, 0:1] if has0 else 1.0
            nc.scalar.activation(S1[:, 0:4, :], pbc[:, :, s1 % 8, :],
                                 AF.Tanh, scale=scl)
            nc.vector.tensor_scalar(S1[:, 0:3, :], S1[:, 0:3, :], 0.5, 0.5,
                                    ALU.mult, ALU.add)
            uv1 = suv.tile([128, 2, BL], F32, name="uv1", tag="uv1")
            nc.vector.tensor_tensor(uv1[:], S1[:, 0:2, :], S1[:, 3:5, :],
                                    ALU.mult)
            nc.vector.tensor_tensor(S1[:, 4, :], uv1[:, 0, :], uv1[:, 1, :],
                                    ALU.add)
            th1 = sth.tile([128, BL], F32, name="th1", tag="th1")
            nc.scalar.activation(th1[:], S1[:, 4, :], AF.Tanh)
            hn1 = shh.tile([128, BL], BF16)
            nc.vector.tensor_tensor(hn1[:], S1[:, 2, :], th1[:], ALU.mult)
            h1_prev = hn1
            if s1 == T - 1:
                h1_last = hn1
    if dbg is not None:
        hl32 = wk.tile([LH, BL], F32)
        nc.scalar.copy(hl32[:], h1_last[:])
        nc.sync.dma_start(dbg[2][:], hl32[:])

    # ================= FC head =================
    p1 = sps.tile([LH // 2, BL], F32, bufs=1)
    nc.tensor.matmul(p1[:], fc1T_t[:], h1_last[:], start=True, stop=True)
    z1 = wk.tile([LH // 2, BL], BF16)
    nc.scalar.activation(z1[:], p1[:], AF.Relu, bias=fc1b_t[:, 0:1], scale=1.0)
    p2 = sps.tile([NC_OUT, BL], F32, bufs=1)
    nc.tensor.matmul(p2[:], fc2T_t[:], z1[:], start=True, stop=True)
    z2o = wk.tile([NC_OUT, BL], F32)
    nc.scalar.activation(z2o[:], p2[:], AF.Identity, bias=fc2b_t[:, 0:1], scale=1.0)
    nc.sync.dma_start(y[:], z2o[:])
    sstack.close()
    stack.close()


# ---------------------------------------------------------------------------
# host packing
# ---------------------------------------------------------------------------
def _pack_inputs(x, adjacency, w1, gamma1, beta1, w2, gamma2, beta2,
                 Wih0, Whh0, bih0, bhh0, Wih1, Whh1, bih1, bhh1,
                 fc1_w, fc1_b, fc2_w, fc2_b):
    x = _f32(x)
    xa_full = np.ascontiguousarray(x.transpose(1, 3, 0, 2))  # [N, F, B, T]

    w1 = _f32(w1); w2 = _f32(w2)
    w1d = np.zeros((2 * FIN, 128), np.float32)
    w2d = np.zeros((128, 128), np.float32)
    for jm in range(2):
        w1d[jm * FIN:(jm + 1) * FIN, jm * 64:(jm + 1) * 64] = w1
        w2d[jm * 64:(jm + 1) * 64, jm * 64:(jm + 1) * 64] = w2

    q = np.arange(128)
    foldm = (q[:, None] % 64 == q[None, :] % 64).astype(np.float32)

    g1rr = np.tile(_f32(gamma1), 2).reshape(128, 1)
    b1rr = np.tile(_f32(beta1), 2).reshape(128, 1)

    W0p = _f32(Wih0)[GPERM]                       # [512, 4096]
    W0r = W0p.reshape(512, 2, 32, 64)             # [g, jm, m', o]
    wih0 = np.ascontiguousarray(
        W0r.transpose(1, 3, 2, 0).reshape(128, 32, 4, 128))
    W0n = W0p.reshape(512, 64, 2, 32)             # [g, n, k, c2']
    wih0b = np.ascontiguousarray(
        W0n.transpose(2, 1, 3, 0).reshape(128, 32, 4, 128))
    fm2 = np.zeros((128, 128), np.float32)
    fm2[:64, :64] = 1.0
    fm2[64:, 64:] = 1.0
    g2 = _f32(gamma2); be2 = _f32(beta2)
    g2kk = np.concatenate([np.tile(g2[:32], (64, 1)), np.tile(g2[32:], (64, 1))])
    b2kk = np.concatenate([np.tile(be2[:32], (64, 1)), np.tile(be2[32:], (64, 1))])
    whh0T = np.ascontiguousarray(_f32(Whh0)[GPERM].T)   # [128, 512]
    # L1 tanh-trick: sigma(x) = 0.5*tanh(x/2)+0.5 -> halve i,f,o rows (0:384
    # in permuted i,f,o,g order); g rows (384:512) stay full for tanh.
    halv = np.concatenate([np.full(384, 0.5, np.float32),
                           np.ones(128, np.float32)])
    wih1T = np.ascontiguousarray((_f32(Wih1)[GPERM] * halv[:, None]).T)
    whh1T = np.ascontiguousarray((_f32(Whh1)[GPERM] * halv[:, None]).T)
    b0 = (_f32(bih0) + _f32(bhh0))[GPERM]
    b0c = np.ascontiguousarray(b0.reshape(4, 128).T)    # [128, 4]
    b1 = (_f32(bih1) + _f32(bhh1))[GPERM] * halv
    b1c = b1.reshape(4, 128).T                          # [128, 4]
    b1t8 = np.ascontiguousarray(np.broadcast_to(
        b1c[:, :, None, None], (128, 4, 8, BL)).copy())

    common = {
        "adj": _f32(adjacency), "eye": np.eye(N, dtype=np.float32),
        "w1d": _bf16(w1d), "w2d": _bf16(w2d), "foldm": foldm,
        "g1r": g1rr, "b1r": b1rr, "g2k": np.ascontiguousarray(g2kk),
        "b2k": np.ascontiguousarray(b2kk), "fm2": fm2,
        "wih0": _bf16(wih0), "wih0b": _bf16(wih0b), "whh0T": _bf16(whh0T),
        "wih1T": _bf16(wih1T), "whh1T": _bf16(whh1T),
        "b0c": b0c, "b1t8": b1t8,
        "fc1T": _bf16(_f32(fc1_w).T), "fc1b": _f32(fc1_b).reshape(-1, 1),
        "fc2T": _bf16(_f32(fc2_w).T), "fc2b": _f32(fc2_b).reshape(-1, 1),
    }
    in_maps = []
    for c in range(NCORES):
        m = dict(common)
        m["xa"] = _bf16(xa_full[:, :, c * BL:(c + 1) * BL, :])
        in_maps.append(m)
    return in_maps


_CACHE = {}


def kernel(**inputs):
    global LAST_EXEC_NS
    debug = bool(int(os.environ.get("STGCN_DEBUG", "0")))
    key = ("dbg" if debug else "std")
    if key not in _CACHE:
        _CACHE[key] = _build(debug=debug)
    nc = _CACHE[key]
    in_maps = _pack_inputs(**inputs)
    kw = {}
    tdir = os.environ.get("STGCN_TRACE_DIR")
    if tdir:
        kw["tmpdir"] = tdir
    res = run_bass_kernel_spmd(nc, in_maps, core_ids=list(range(NCORES)), **kw)
    LAST_EXEC_NS = res.exec_time_ns
    if debug:
        kernel.debug_results = res.results
    out = np.zeros((B, NC_OUT), np.float32)
    for c in range(NCORES):
        out[c * BL:(c + 1) * BL, :] = np.asarray(res.results[c]["y"], np.float32).T
    return out



# revision 40
# speedup vs baseline: 1.0484x; 1.0012x over previous
"""STGCN fully on-device for 8 Trainium2 NeuronCores.

Data-parallel over batch (4 examples/core). Entire forward runs in ONE Bass
SPMD launch per call:
  - adjacency normalization on device
  - GCN1: A-first matmul (thin), transpose, W-matmul (block-diag packed,
    128-partition), BN1 stats via bn_stats + cross-core AllReduce, fused
    BN-apply+ReLU on ACT
  - GCN2: W-first matmul, DMA transpose, A-matmul (block-diag), DMA
    transpose back, BN2 (same path), residual add
  - LSTM0+LSTM1 pipelined scan: 129 slots, PSUM-preloaded xp/bias, 4+8
    matmuls/slot, fused sigmoid/tanh across both layers
  - FC head, output DMA

Layout glossary (per core, BL=4):
  jm in {0,1} node-half, node n = jm*32+m' ; channel o in [0,64)
  "L-layout":  [p=(jm,o)=128, (m'=32, b=4, t=128)]
  gates order: i, f, o, g  (PyTorch i,f,g,o permuted so sigmoid gates are
  adjacent)
"""
import os

import numpy as np
import ml_dtypes

import concourse.bass as bass
import concourse.tile as tile
from concourse import mybir
from concourse.bass_utils import run_bass_kernel_spmd

F32 = mybir.dt.float32
BF16 = mybir.dt.bfloat16
AF = mybir.ActivationFunctionType
ALU = mybir.AluOpType

B, N, T, FIN = 32, 64, 128, 9
GH, LH, NC_OUT = 64, 128, 16
NCORES = 8
BL = B // NCORES          # 4 examples per core
EPS = 1e-5
S_TOT = float(B * N * T)  # BN sample count (full batch)

# gate permutation: torch order i,f,g,o -> i,f,o,g
GPERM = np.concatenate([np.arange(0, 128), np.arange(128, 256),
                        np.arange(384, 512), np.arange(256, 384)])

LAST_EXEC_NS = None


def _bf16(a):
    return np.ascontiguousarray(np.asarray(a, np.float32).astype(ml_dtypes.bfloat16))


def _f32(a):
    return np.ascontiguousarray(np.asarray(a, np.float32))


# ---------------------------------------------------------------------------
# walrus wait-cap workaround (same as baseline)
# ---------------------------------------------------------------------------
def _split_excess_waits(nc):
    fix_id = 0
    for fn in nc.m.functions:
        for blk in fn.blocks:
            out = []
            changed = False
            for inst in blk.instructions:
                si = inst.sync_info
                waits = list(si.on_wait) if si and si.on_wait else []
                cap = 2 if isinstance(inst, mybir.InstEventSemaphore) else 1
                if len(waits) > cap:
                    extra, keep = waits[: len(waits) - cap], waits[len(waits) - cap:]
                    for w in extra:
                        nop = mybir.InstNoOp(name=f"waitfix-{fix_id}")
                        fix_id += 1
                        nop.engine = inst.engine
                        nop.sync_info = mybir.SyncInfo(on_wait=[w], on_update=[])
                        nop.debug = inst.debug
                        nc.register_instruction(nop, overwrite=True)
                        out.append(nop)
                    si.on_wait = keep
                    changed = True
                out.append(inst)
            if changed:
                blk.instructions = out
    return nc


# ---------------------------------------------------------------------------
# device program
# ---------------------------------------------------------------------------
def _build(debug=False):
    nc = bass.Bass(num_devices=NCORES)
    d = {}

    def din(name, shape, dt):
        d[name] = nc.dram_tensor(name, shape, dt, kind="ExternalInput")
        return d[name]

    xa = din("xa", [N, FIN, BL, T], BF16)          # x for A-mm1: [n,(f,b,t)]
    adj = din("adj", [N, N], F32)
    eye = din("eye", [N, N], F32)
    w1d = din("w1d", [2 * FIN, 128], BF16)         # blockdiag(w1,w1)
    w2d = din("w2d", [128, 128], BF16)             # blockdiag over jm of w2
    foldm = din("foldm", [128, 128], F32)          # mod-64 partition fold
    g1r = din("g1r", [128, 1], F32)
    b1r = din("b1r", [128, 1], F32)
    g2k = din("g2k", [128, 32], F32)               # gamma2 in (k,c2') layout
    b2k = din("b2k", [128, 32], F32)
    wih0 = din("wih0", [128, 32, 4, 128], BF16)    # [r=(jm,o), kchunk=m', kg, grow]
    wih0b = din("wih0b", [128, 32, 4, 128], BF16)  # [r=(k,m), kchunk=c2', kg, grow]
    fm2 = din("fm2", [128, 128], F32)              # blockdiag(ones64) fold
    whh0T = din("whh0T", [LH, 4 * LH], BF16)
    wih1T = din("wih1T", [LH, 4 * LH], BF16)
    whh1T = din("whh1T", [LH, 4 * LH], BF16)
    b0c = din("b0c", [128, 4], F32)                # bias0 per gate-chunk col
    b1t8 = din("b1t8", [128, 4, 8, BL], F32)       # L1 block-psum preload
    fc1T = din("fc1T", [LH, LH // 2], BF16)
    fc1b = din("fc1b", [LH // 2, 1], F32)
    fc2T = din("fc2T", [LH // 2, NC_OUT], BF16)
    fc2b = din("fc2b", [NC_OUT, 1], F32)
    y = nc.dram_tensor("y", [NC_OUT, BL], F32, kind="ExternalOutput")
    if debug:
        dbg_xp0 = nc.dram_tensor("dbg_xp0", [128, 4, BL, T], F32, kind="ExternalOutput")
        dbg_h0 = nc.dram_tensor("dbg_h0", [LH, BL], F32, kind="ExternalOutput")

    with tile.TileContext(nc) as tc:
        _prog(nc, tc, d, y,
              dbg=(None, dbg_xp0, dbg_h0) if debug else None)
    _split_excess_waits(nc)
    return nc


def _bn_block(nc, tc, wk, st, gr, br, dram, foldm_t, tag):
    """Aggregate precomputed bn_stats `st` [128,32,6] -> cross-core allreduce
    -> per-partition scale/bias [128,1] f32. Returns (sc, bs)."""
    mv = wk.tile([128, 2], F32)
    nc.vector.bn_aggr(mv[:], st[:])
    # local sums: n_loc = 32*BL*T per partition
    n_loc = float(32 * BL * T)
    sums = wk.tile([128, 2], F32)
    # sums[:,0] = mean*n_loc ; sums[:,1] = (var + mean^2)*n_loc
    msq = wk.tile([128, 1], F32)
    nc.vector.tensor_tensor(msq[:], mv[:, 0:1], mv[:, 0:1], ALU.mult)
    nc.vector.tensor_tensor(sums[:, 1:2], mv[:, 1:2], msq[:], ALU.add)
    nc.scalar.mul(sums[:, 0:1], mv[:, 0:1], n_loc)
    nc.scalar.mul(sums[:, 1:2], sums[:, 1:2], n_loc)
    # allreduce over 8 cores
    bi = dram.tile([128, 2], F32)
    bo = dram.tile([128, 2], F32)
    nc.sync.dma_start(bi[:], sums[:])
    nc.gpsimd.collective_compute(
        "AllReduce", ALU.add,
        replica_groups=[list(range(NCORES))],
        ins=[bi[:].opt()], outs=[bo[:].opt()])
    red = wk.tile([128, 2], F32)
    nc.sync.dma_start(red[:], bo[:])
    # fold jm-halves (mod-64) via PE: out[p,:] = sum_q foldm[q,p]*red[q,:]
    with tc.tile_pool(name=f"bnps{tag}", bufs=1, space="PSUM") as ps:
        pf = ps.tile([128, 2], F32)
        nc.tensor.matmul(pf[:], foldm_t[:], red[:], start=True, stop=True)
        tot = wk.tile([128, 2], F32)
        nc.scalar.mul(tot[:], pf[:], 1.0 / S_TOT)   # [mean, E[x^2]]
    var = wk.tile([128, 1], F32)
    m2 = wk.tile([128, 1], F32)
    nc.vector.tensor_tensor(m2[:], tot[:, 0:1], tot[:, 0:1], ALU.mult)
    nc.vector.tensor_tensor(var[:], tot[:, 1:2], m2[:], ALU.subtract)
    epst = wk.tile([128, 1], F32)
    nc.vector.memset(epst[:], EPS)
    sd = wk.tile([128, 1], F32)
    nc.scalar.activation(sd[:], var[:], AF.Sqrt, bias=epst[:, 0:1], scale=1.0)
    rs = wk.tile([128, 1], F32)
    nc.vector.reciprocal(rs[:], sd[:])
    sc = wk.tile([128, 1], F32)
    nc.vector.tensor_tensor(sc[:], rs[:], gr[:], ALU.mult)
    mscale = wk.tile([128, 1], F32)
    nc.vector.tensor_tensor(mscale[:], tot[:, 0:1], sc[:], ALU.mult)
    bs = wk.tile([128, 1], F32)
    nc.vector.tensor_tensor(bs[:], br[:], mscale[:], ALU.subtract)
    return sc, bs


def _prog(nc, tc, d, y, dbg=None):
    from contextlib import ExitStack
    stack = ExitStack()
    wk = stack.enter_context(tc.tile_pool(name="wk", bufs=1))       # persistent small
    dram = stack.enter_context(tc.tile_pool(name="drb", bufs=1, space="DRAM"))

    # ---------------- warm up the collective fabric (absorbs comm init +
    # launch skew off the BN1 allreduce's critical path)
    warm = wk.tile([128, 1], F32, name="warm")
    nc.vector.memset(warm[:], 0.0)
    wi = dram.tile([128, 1], F32, name="wi")
    wo = dram.tile([128, 1], F32, name="wo")
    nc.gpsimd.dma_start(wi[:], warm[:])
    nc.gpsimd.collective_compute(
        "AllReduce", ALU.add,
        replica_groups=[list(range(NCORES))],
        ins=[wi[:].opt()], outs=[wo[:].opt()])

    # ---------------- critical-path inputs first: xa + adjacency + w1d
    LEFT, RIGHT = "left", "right"
    pool_L = tc.alloc_tile_pool(name="pL", bufs=1, side=LEFT)    # x2 (long)
    pool_A = tc.alloc_tile_pool(name="pA", bufs=1, side=LEFT)    # xa, g1
    adj_t = wk.tile([N, N], F32, name="adj_t")
    eye_t = wk.tile([N, N], F32, name="eye_t")
    xa_t = pool_A.tile([N, FIN, BL, T], BF16, name="xa_t")
    w1d_t = wk.tile([2 * FIN, 128], BF16, name="w1d_t")
    with tc.high_priority():
        nc.sync.dma_start(xa_t[:, 0:3, :, :], d["xa"][:, 0:3, :, :])
        nc.scalar.dma_start(xa_t[:, 3:6, :, :], d["xa"][:, 3:6, :, :])
        nc.gpsimd.dma_start(xa_t[:, 6:9, :, :], d["xa"][:, 6:9, :, :])
        nc.sync.dma_start(adj_t[:], d["adj"][:])
        nc.sync.dma_start(eye_t[:], d["eye"][:])
        nc.sync.dma_start(w1d_t[:], d["w1d"][:])

    # ---------------- PE p-state warmup (reach 2.4GHz before GCN matmuls)
    wz = wk.tile([128, 512], BF16, name="wz")
    nc.vector.memset(wz[:], 0.0)
    with tc.tile_pool(name="wups", bufs=1, space="PSUM") as wps:
        wp = wps.tile([128, 512], F32, name="wp")
        for _ in range(8):
            nc.tensor.matmul(wp[:], wz[:, 0:128], wz[:], start=True, stop=True)

    # ---------------- remaining small constants (gpsimd queue: keeps the
    # sync ring free for xa/adj so A-mm1 can start early)
    foldm_t = wk.tile([128, 128], F32, name="foldm_t")
    nc.gpsimd.dma_start(foldm_t[:], d["foldm"][:])
    g1r = wk.tile([128, 1], F32, name="g1r")
    nc.gpsimd.dma_start(g1r[:], d["g1r"][:])
    b1r = wk.tile([128, 1], F32, name="b1r")
    nc.gpsimd.dma_start(b1r[:], d["b1r"][:])
    g2k = wk.tile([128, 32], F32, name="g2k")
    nc.gpsimd.dma_start(g2k[:], d["g2k"][:])
    b2k = wk.tile([128, 32], F32, name="b2k")
    nc.gpsimd.dma_start(b2k[:], d["b2k"][:])
    fm2_t = wk.tile([128, 128], F32, name="fm2_t")
    nc.gpsimd.dma_start(fm2_t[:], d["fm2"][:])
    w2d_t = wk.tile([128, 128], BF16, name="w2d_t")
    nc.gpsimd.dma_start(w2d_t[:], d["w2d"][:])

    a1 = wk.tile([N, N], F32)
    nc.vector.tensor_tensor(a1[:], adj_t[:], eye_t[:], ALU.add)
    deg = wk.tile([N, 1], F32)
    nc.vector.tensor_reduce(deg[:], a1[:], mybir.AxisListType.X, ALU.add)
    sdg = wk.tile([N, 1], F32)
    nc.scalar.activation(sdg[:], deg[:], AF.Sqrt)
    dinv = wk.tile([N, 1], F32)
    nc.vector.reciprocal(dinv[:], sdg[:])
    a2 = wk.tile([N, N], F32)
    nc.vector.tensor_scalar(a2[:], a1[:], dinv[:], None, ALU.mult)
    ones1 = wk.tile([1, N], F32)
    nc.vector.memset(ones1[:], 1.0)
    with tc.tile_pool(name="adjps", bufs=1, space="PSUM") as ps:
        pdt = ps.tile([1, N], F32)
        nc.tensor.transpose(pdt[:], dinv[:], eye_t[:])
        dT = wk.tile([1, N], F32)
        nc.scalar.copy(dT[:], pdt[:])
        pbc = ps.tile([N, N], F32)
        nc.tensor.matmul(pbc[:], ones1[:], dT[:], start=True, stop=True)
        ah = wk.tile([N, N], BF16)       # normalized adjacency, bf16
        nc.vector.tensor_tensor(ah[:], a2[:], pbc[:], ALU.mult)
    ahd = wk.tile([128, 128], BF16)      # blockdiag(ah, ah)
    nc.vector.memset(ahd[:], 0.0)
    nc.scalar.copy(ahd[0:64, 0:64], ah[:])
    nc.scalar.copy(ahd[64:128, 64:128], ah[:])

    # ---------------- LSTM weights; the two big 4MB tiles (wih0/wih0b) are
    # triggered AFTER A-mm1 below so their HBM traffic cannot delay xa/adj.
    wih0_t = wk.tile([128, 32, 4, 128], BF16, name="wih0_t")
    wih0b_t = wk.tile([128, 32, 4, 128], BF16, name="wih0b_t")
    whh0T_t = wk.tile([LH, 512], BF16, name="whh0T_t")
    nc.scalar.dma_start(whh0T_t[:], d["whh0T"][:])
    wih1T_t = wk.tile([LH, 512], BF16, name="wih1T_t")
    nc.scalar.dma_start(wih1T_t[:], d["wih1T"][:])
    whh1T_t = wk.tile([LH, 512], BF16, name="whh1T_t")
    nc.scalar.dma_start(whh1T_t[:], d["whh1T"][:])
    b0c_t = wk.tile([128, 4], F32, name="b0c_t")
    nc.sync.dma_start(b0c_t[:], d["b0c"][:])
    b1t8_t = wk.tile([128, 4, 8, BL], F32, name="b1t8_t")
    nc.sync.dma_start(b1t8_t[:], d["b1t8"][:])
    fc1T_t = wk.tile([LH, LH // 2], BF16, name="fc1T_t")
    nc.sync.dma_start(fc1T_t[:], d["fc1T"][:])
    fc1b_t = wk.tile([LH // 2, 1], F32, name="fc1b_t")
    nc.sync.dma_start(fc1b_t[:], d["fc1b"][:])
    fc2T_t = wk.tile([LH // 2, NC_OUT], BF16, name="fc2T_t")
    nc.sync.dma_start(fc2T_t[:], d["fc2T"][:])
    fc2b_t = wk.tile([NC_OUT, 1], F32, name="fc2b_t")
    nc.sync.dma_start(fc2b_t[:], d["fc2b"][:])

    # ================= GCN =================
    # Phase pools: LIFO per side; big tensors phase-scoped to fit SBUF.
    ps_cm = tc.tile_pool(name="gps", bufs=4, space="PSUM")
    ps = ps_cm.__enter__()
    pool_B = tc.alloc_tile_pool(name="pB", bufs=1, side=RIGHT)   # g1p

    # --- A-mm1: G1[m,(f,b,t)] = ah @ xa
    g1 = pool_A.tile([N, FIN, BL, T], BF16, name="g1")
    for fc in range(FIN):
        p = ps.tile([N, BL, T], F32, name="p_amm1", tag="mmp")
        nc.tensor.matmul(p[:], ah[:], xa_t[:, fc, :, :], start=True, stop=True)
        if fc % 2 == 0:
            nc.scalar.copy(g1[:, fc, :, :], p[:])
        else:
            nc.vector.tensor_copy(g1[:, fc, :, :], p[:])
    # big LSTM input-projection weights: start their HBM pulls now
    nc.scalar.dma_start(wih0_t[:], d["wih0"][:])
    nc.scalar.dma_start(wih0b_t[:], d["wih0b"][:])
    # --- thin transpose: G1 -> G1p [(jm,f), (m',b,t)]
    g1p = pool_B.tile([2 * FIN, 32, BL, T], BF16, name="g1p")
    for mp in range(32):
        # dst [(jm,f), b, t] <- src g1[{mp, 32+mp}, f, b, t]
        eng = [nc.sync, nc.scalar, nc.gpsimd][mp % 3]
        eng.dma_start(g1p[:, mp, :, :], g1[mp::32, :, :, :])
    pool_A.release()
    pool_C = tc.alloc_tile_pool(name="pC", bufs=1, side=LEFT)    # h1
    # --- W-mm1: H1[(jm,o),(m',b,t)]; copies on ACT, bn_stats inline on DVE
    # so the BN1 allreduce can fire the moment the last chunk lands.
    h1 = pool_C.tile([128, 32, BL, T], BF16, name="h1")
    st1 = wk.tile([128, 32, 6], F32, name="st1")
    for c in range(32):
        p = ps.tile([128, BL, T], F32, name="p_wmm1", tag="mmp")
        nc.tensor.matmul(p[:], w1d_t[:], g1p[:, c, :, :], start=True, stop=True)
        nc.scalar.copy(h1[:, c, :, :], p[:])
        nc.vector.bn_stats(st1[:, c, :], h1[:, c, :, :].opt())
    pool_B.release()
    pool_G = tc.alloc_tile_pool(name="pG", bufs=1, side=RIGHT)   # h2 (hs)
    h2 = pool_G.tile([128, 32, BL, T], BF16, name="h2")
    # --- BN1
    sc1, bs1 = _bn_block(nc, tc, wk, st1, g1r, b1r, dram, foldm_t, "1")
    x2 = pool_L.tile([128, 32, BL, T], BF16, name="x2")
    for c in range(32):
        if c % 2 == 0:
            nc.scalar.activation(x2[:, c, :, :], h1[:, c, :, :], AF.Relu,
                                 bias=bs1[:], scale=sc1[:])
        else:
            nc.vector.tensor_scalar(x2[:, c, :, :], h1[:, c, :, :],
                                    sc1[:, 0:1], bs1[:, 0:1],
                                    ALU.mult, ALU.add)
            nc.vector.tensor_scalar_max(x2[:, c, :, :], x2[:, c, :, :], 0.0)
    pool_C.release()
    pool_E = tc.alloc_tile_pool(name="pE", bufs=1, side=LEFT)    # z2p
    # ================= GCN layer 2 =================
    # W-mm2 per chunk -> staging -> scatter-DMA directly into transposed Z2p.
    # Z2 chunk [(jm,c2),(b,t)] scatters to Z2p [(k,n),(c2',b,t)], c2=k*32+c2'.
    z2p = pool_E.tile([128, 32, BL, T], BF16, name="z2p")
    with tc.tile_pool(name="stg2", bufs=10) as stg2p:
        for c in range(32):
            p = ps.tile([128, BL, T], F32, name="p_wmm2", tag="mmp")
            nc.tensor.matmul(p[:], w2d_t[:], x2[:, c, :, :], start=True, stop=True)
            stg = stg2p.tile([128, BL, T], BF16, name="stg")
            nc.vector.tensor_copy(stg[:], p[:])
            for jm in range(2):
                eng = [nc.sync, nc.gpsimd, nc.scalar][(2 * c + jm) % 3]
                # dst partitions {k*64+jm*32+c : k in 0,1}; iter (k,(c2',b,t))
                eng.dma_start(z2p[jm * 32 + c::64, :, :, :],
                              stg[jm * 64:(jm + 1) * 64, :, :])
    # ====== xp0 (x2-half, kg 0/1): fills PE while z2p scatter transfers land
    xps_cm = tc.tile_pool(name="xps", bufs=1, space="PSUM")
    xps = xps_cm.__enter__()
    xp_ps = {}
    for kg in (0, 1):
        pxp = xps.tile([128, BL, T], F32, name=f"p_xp{kg}")
        for c in range(32):
            nc.tensor.matmul(pxp[:], wih0_t[:, c, kg, :], x2[:, c, :, :],
                             start=(c == 0), stop=False)
        xp_ps[kg] = pxp
    # --- A-mm2: H2 [(k,m),(c2',b,t)]; BN2 runs in THIS layout (no transpose
    # back): per-chunk c2' the channel is fixed per partition-half, so ACT
    # per-partition scale/bias still works with [128,32] scale tiles.
    st2 = wk.tile([128, 32, 6], F32, name="st2")
    for c in range(32):
        p = ps.tile([128, BL, T], F32, name="p_amm2", tag="mmp")
        nc.tensor.matmul(p[:], ahd[:], z2p[:, c, :, :], start=True, stop=True)
        nc.scalar.copy(h2[:, c, :, :], p[:])
        nc.vector.bn_stats(st2[:, c, :], h2[:, c, :, :].opt())
    pool_E.release()
    # --- BN2 stats: per (partition,(k,m)), chunk c2' -> fold over m -> allreduce
    mv2 = wk.tile([128, 32, 2], F32, name="mv2")
    for c in range(32):
        nc.vector.bn_aggr(mv2[:, c, :], st2[:, c, :])
    n2 = float(BL * T)
    sums2 = wk.tile([128, 32, 2], F32, name="sums2")
    msq2 = wk.tile([128, 32], F32, name="msq2")
    nc.vector.tensor_tensor(msq2[:], mv2[:, :, 0], mv2[:, :, 0], ALU.mult)
    nc.vector.tensor_tensor(sums2[:, :, 1], mv2[:, :, 1], msq2[:], ALU.add)
    nc.scalar.mul(sums2[:, :, 0], mv2[:, :, 0], n2)
    nc.scalar.mul(sums2[:, :, 1], sums2[:, :, 1], n2)
    # fold over m-partitions within each k-half: out[p=(k,*)] = sum_m sums2[(k,m)]
    bi2 = dram.tile([128, 64], F32, name="bi2")
    bo2 = dram.tile([128, 64], F32, name="bo2")
    pf2 = ps.tile([128, 64], F32, name="pf2", tag="mmp")
    nc.tensor.matmul(pf2[:], fm2_t[:], sums2[:].opt(), start=True, stop=True)
    folded = wk.tile([128, 32, 2], F32, name="folded")
    nc.scalar.copy(folded[:], pf2[:])
    nc.sync.dma_start(bi2[:], folded[:])
    nc.gpsimd.collective_compute(
        "AllReduce", ALU.add,
        replica_groups=[list(range(NCORES))],
        ins=[bi2[:].opt()], outs=[bo2[:].opt()])
    # ====== xp0 (x2-half, kg 2/3): fills the BN2-allreduce + apply window
    for kg in (2, 3):
        pxp = xps.tile([128, BL, T], F32, name=f"p_xp{kg}")
        for c in range(32):
            nc.tensor.matmul(pxp[:], wih0_t[:, c, kg, :], x2[:, c, :, :],
                             start=(c == 0), stop=False)
        xp_ps[kg] = pxp
    red2 = wk.tile([128, 32, 2], F32, name="red2")
    nc.sync.dma_start(red2[:], bo2[:])
    tot2 = wk.tile([128, 32, 2], F32, name="tot2")
    nc.scalar.mul(tot2[:], red2[:], 1.0 / S_TOT)
    var2 = wk.tile([128, 32], F32, name="var2")
    m22 = wk.tile([128, 32], F32, name="m22")
    nc.vector.tensor_tensor(m22[:], tot2[:, :, 0], tot2[:, :, 0], ALU.mult)
    nc.vector.tensor_tensor(var2[:], tot2[:, :, 1], m22[:], ALU.subtract)
    eps2 = wk.tile([128, 32], F32, name="eps2")
    nc.vector.memset(eps2[:], EPS)
    vpe = wk.tile([128, 32], F32, name="vpe")
    nc.vector.tensor_tensor(vpe[:], var2[:], eps2[:], ALU.add)
    sd2 = wk.tile([128, 32], F32, name="sd2")
    nc.scalar.activation(sd2[:], vpe[:], AF.Sqrt)
    rs2 = wk.tile([128, 32], F32, name="rs2")
    nc.vector.reciprocal(rs2[:], sd2[:])
    scf = wk.tile([128, 32], F32, name="scf")
    nc.vector.tensor_tensor(scf[:], rs2[:], g2k[:], ALU.mult)
    mscf = wk.tile([128, 32], F32, name="mscf")
    nc.vector.tensor_tensor(mscf[:], tot2[:, :, 0], scf[:], ALU.mult)
    bsf = wk.tile([128, 32], F32, name="bsf")
    nc.vector.tensor_tensor(bsf[:], b2k[:], mscf[:], ALU.subtract)
    # --- BN2-apply + relu in place on h2 (becomes X3 in (k,m)-layout)
    for c in range(32):
        if c % 2 == 0:
            nc.scalar.activation(h2[:, c, :, :], h2[:, c, :, :], AF.Relu,
                                 bias=bsf[:, c:c + 1], scale=scf[:, c:c + 1])
        else:
            nc.vector.tensor_scalar(h2[:, c, :, :], h2[:, c, :, :],
                                    scf[:, c:c + 1], bsf[:, c:c + 1],
                                    ALU.mult, ALU.add)
            nc.vector.tensor_scalar_max(h2[:, c, :, :], h2[:, c, :, :], 0.0)
    # ====== xp0 second half: += Wih0b @ X3 + b0 (residual folded into mm) ======
    xp0 = wk.tile([128, 4, BL, T], F32, name="xp0")
    for kg in range(4):
        pxp = xp_ps[kg]
        for c in range(32):
            nc.tensor.matmul(pxp[:], wih0b_t[:, c, kg, :], h2[:, c, :, :],
                             start=False, stop=(c == 31))
        nc.scalar.activation(xp0[:, kg, :, :], pxp[:], AF.Identity,
                             bias=b0c_t[:, kg:kg + 1], scale=1.0)
    xps_cm.__exit__(None, None, None)
    if dbg is not None:
        nc.sync.dma_start(dbg[1][:], xp0[:])
    pool_G.release()
    pool_L.release()
    ps_cm.__exit__(None, None, None)

    # ================= LSTM scan =================
    # L0 unchanged (4 whh0 mm + 3 ACT + 4 DVE per slot, high-prio chain).
    # L1 revamped:
    #   - wih1 @ h0 precomputed in 8-step blocks (4 matmuls per 8 slots into
    #     a held PSUM block preloaded with b1), so slots carry only whh1's 4.
    #   - tanh-only gates: i,f,o rows of Wih1/Whh1/b1 pre-halved on host, so
    #     ONE Tanh ACT covers all 4 gates; sigma = 0.5*tanh+0.5 on DVE.
    #   - L1 lags L0 by LAG slots (needs the h0 block complete).
    LAG = 9
    sstack = ExitStack()
    sps = sstack.enter_context(tc.tile_pool(name="sps", bufs=3, space="PSUM"))
    bps = sstack.enter_context(tc.tile_pool(name="bps", bufs=2, space="PSUM"))
    sgt = sstack.enter_context(tc.tile_pool(name="sgt", bufs=3))
    suv = sstack.enter_context(tc.tile_pool(name="suv", bufs=3))
    sth = sstack.enter_context(tc.tile_pool(name="sth", bufs=3))
    shh = sstack.enter_context(tc.tile_pool(name="shh", bufs=3))
    sst = sstack.enter_context(tc.tile_pool(name="sst", bufs=1))

    S0 = sst.tile([128, 2, BL], F32, name="S0")    # [tanh(g), c] for L0
    S1 = sst.tile([128, 5, BL], F32, name="S1")    # [i,f,o,g,c] for L1
    nc.vector.memset(S0[:], 0.0)
    nc.vector.memset(S1[:], 0.0)
    hbuf0 = sst.tile([128, 2, 8, BL], BF16, name="hbuf0")  # h0 ring (2 blocks)
    gate_tok = sst.tile([128, 1], F32, name="gate_tok")
    nc.vector.memset(gate_tok[:], 1.0)
    pb_blocks = {}
    h1_prev = None
    h1_last = None
    for t in range(T + LAG):
        has0 = t < T
        s1 = t - LAG
        has1 = 0 <= s1 < T
        # --- L1 input-projection block: pb[k,s',b] = b1 + wih1 @ h0[8j+s']
        if t % 8 == 0 and 8 <= t <= T:
            j = t // 8 - 1
            pb = bps.tile([128, 4, 8, BL], F32, name="pb", tag="pb")
            nc.vector.tensor_copy(pb[:], b1t8_t[:])
            for k in range(4):
                nc.tensor.matmul(pb[:, k, :, :],
                                 wih1T_t[:, k * LH:(k + 1) * LH],
                                 hbuf0[:, j % 2, :, :],
                                 start=False, stop=True, skip_group_check=True)
            pb_blocks[j] = pb
        # --- L0 psum preload + recurrent matmuls (critical path)
        if has0:
            pt0 = sps.tile([128, 4 * BL], F32, name="pt0", tag="pt0", bufs=3)
            nc.vector.tensor_copy(pt0[:], xp0[:, :, :, t])
            if t >= 1:
                hp0 = hbuf0[:, ((t - 1) // 8) % 2, (t - 1) % 8, :]
                with tc.high_priority(offset=30):
                    # g-gate first so tanh(g) can start while i,f,o stream
                    for k in (3, 0, 1, 2):
                        nc.tensor.matmul(pt0[:, k * BL:(k + 1) * BL],
                                         whh0T_t[:, k * LH:(k + 1) * LH],
                                         hp0,
                                         start=False, stop=True,
                                         skip_group_check=True)
        # --- L1 recurrent matmuls into its block slot (demoted: they become
        # ready only mid-slot, so sort them behind the next L0 chain)
        if has1 and s1 >= 1:
            pbc = pb_blocks[s1 // 8]
            with tc.high_priority(offset=-45):
                for k in range(4):
                    nc.tensor.matmul(pbc[:, k, s1 % 8, :],
                                     whh1T_t[:, k * LH:(k + 1) * LH],
                                     h1_prev[:],
                                     start=False, stop=True,
                                     skip_group_check=True)
        # --- L0 cell update
        if has0:
            with tc.high_priority(offset=30):
                gt0 = sgt.tile([128, 3, BL], F32)
                # tanh(g) first: its matmul lands first, so it overlaps the
                # sigmoid instead of serializing after it on ACT
                nc.scalar.activation(S0[:, 0, :], pt0[:, 3 * BL:4 * BL], AF.Tanh)
                nc.scalar.activation(gt0[:], pt0[:, 0:3 * BL], AF.Sigmoid)
                uv0 = suv.tile([128, 2, BL], F32)
                nc.vector.tensor_tensor(uv0[:], gt0[:, 0:2, :], S0[:], ALU.mult)
                nc.vector.tensor_tensor(S0[:, 1, :], uv0[:, 0, :], uv0[:, 1, :],
                                        ALU.add)
                th0 = sth.tile([128, BL], F32)
                nc.scalar.activation(th0[:], S0[:, 1, :], AF.Tanh)
                nc.vector.tensor_tensor(hbuf0[:, (t // 8) % 2, t % 8, :],
                                        gt0[:, 2, :], th0[:], ALU.mult)
                # gate_tok = th0*0 + 1 : data-dep marker ordering L1 after L0
                # (issued after hn0 so it never sits on the recurrence chain)
                nc.vector.tensor_scalar(gate_tok[:], th0[:, 0:1], 0.0, 1.0,
                                        ALU.mult, ALU.add)
        # --- L1 cell update (tanh-trick), demoted with its matmuls
        if has1:
            pbc = pb_blocks[s1 // 8]
            scl = gate_tok[:, 0:1] if has0 else 1.0
            with tc.high_priority(offset=-45):
                nc.scalar.activation(S1[:, 0:4, :], pbc[:, :, s1 % 8, :],
                                     AF.Tanh, scale=scl)
                nc.vector.tensor_scalar(S1[:, 0:3, :], S1[:, 0:3, :], 0.5, 0.5,
                                        ALU.mult, ALU.add)
                uv1 = suv.tile([128, 2, BL], F32, name="uv1", tag="uv1")
                nc.vector.tensor_tensor(uv1[:], S1[:, 0:2, :], S1[:, 3:5, :],
                                        ALU.mult)
                nc.vector.tensor_tensor(S1[:, 4, :], uv1[:, 0, :], uv1[:, 1, :],
                                        ALU.add)
                th1 = sth.tile([128, BL], F32, name="th1", tag="th1")
                nc.scalar.activation(th1[:], S1[:, 4, :], AF.Tanh)
                hn1 = shh.tile([128, BL], BF16)
                nc.vector.tensor_tensor(hn1[:], S1[:, 2, :], th1[:], ALU.mult)
            h1_prev = hn1
            if s1 == T - 1:
                h1_last = hn1
    if dbg is not None:
        hl32 = wk.tile([LH, BL], F32)
        nc.scalar.copy(hl32[:], h1_last[:])
        nc.sync.dma_start(dbg[2][:], hl32[:])

    # ================= FC head =================
    p1 = sps.tile([LH // 2, BL], F32, bufs=1)
    nc.tensor.matmul(p1[:], fc1T_t[:], h1_last[:], start=True, stop=True)
    z1 = wk.tile([LH // 2, BL], BF16)
    nc.scalar.activation(z1[:], p1[:], AF.Relu, bias=fc1b_t[:, 0:1], scale=1.0)
    p2 = sps.tile([NC_OUT, BL], F32, bufs=1)
    nc.tensor.matmul(p2[:], fc2T_t[:], z1[:], start=True, stop=True)
    z2o = wk.tile([NC_OUT, BL], F32)
    nc.scalar.activation(z2o[:], p2[:], AF.Identity, bias=fc2b_t[:, 0:1], scale=1.0)
    nc.sync.dma_start(y[:], z2o[:])
    sstack.close()
    stack.close()


# ---------------------------------------------------------------------------
# host packing
# ---------------------------------------------------------------------------
def _pack_inputs(x, adjacency, w1, gamma1, beta1, w2, gamma2, beta2,
                 Wih0, Whh0, bih0, bhh0, Wih1, Whh1, bih1, bhh1,
                 fc1_w, fc1_b, fc2_w, fc2_b):
    x = _f32(x)
    xa_full = np.ascontiguousarray(x.transpose(1, 3, 0, 2))  # [N, F, B, T]

    w1 = _f32(w1); w2 = _f32(w2)
    w1d = np.zeros((2 * FIN, 128), np.float32)
    w2d = np.zeros((128, 128), np.float32)
    for jm in range(2):
        w1d[jm * FIN:(jm + 1) * FIN, jm * 64:(jm + 1) * 64] = w1
        w2d[jm * 64:(jm + 1) * 64, jm * 64:(jm + 1) * 64] = w2

    q = np.arange(128)
    foldm = (q[:, None] % 64 == q[None, :] % 64).astype(np.float32)

    g1rr = np.tile(_f32(gamma1), 2).reshape(128, 1)
    b1rr = np.tile(_f32(beta1), 2).reshape(128, 1)

    W0p = _f32(Wih0)[GPERM]                       # [512, 4096]
    W0r = W0p.reshape(512, 2, 32, 64)             # [g, jm, m', o]
    wih0 = np.ascontiguousarray(
        W0r.transpose(1, 3, 2, 0).reshape(128, 32, 4, 128))
    W0n = W0p.reshape(512, 64, 2, 32)             # [g, n, k, c2']
    wih0b = np.ascontiguousarray(
        W0n.transpose(2, 1, 3, 0).reshape(128, 32, 4, 128))
    fm2 = np.zeros((128, 128), np.float32)
    fm2[:64, :64] = 1.0
    fm2[64:, 64:] = 1.0
    g2 = _f32(gamma2); be2 = _f32(beta2)
    g2kk = np.concatenate([np.tile(g2[:32], (64, 1)), np.tile(g2[32:], (64, 1))])
    b2kk = np.concatenate([np.tile(be2[:32], (64, 1)), np.tile(be2[32:], (64, 1))])
    whh0T = np.ascontiguousarray(_f32(Whh0)[GPERM].T)   # [128, 512]
    # L1 tanh-trick: sigma(x) = 0.5*tanh(x/2)+0.5 -> halve i,f,o rows (0:384
    # in permuted i,f,o,g order); g rows (384:512) stay full for tanh.
    halv = np.concatenate([np.full(384, 0.5, np.float32),
                           np.ones(128, np.float32)])
    wih1T = np.ascontiguousarray((_f32(Wih1)[GPERM] * halv[:, None]).T)
    whh1T = np.ascontiguousarray((_f32(Whh1)[GPERM] * halv[:, None]).T)
    b0 = (_f32(bih0) + _f32(bhh0))[GPERM]
    b0c = np.ascontiguousarray(b0.reshape(4, 128).T)    # [128, 4]
    b1 = (_f32(bih1) + _f32(bhh1))[GPERM] * halv
    b1c = b1.reshape(4, 128).T                          # [128, 4]
    b1t8 = np.ascontiguousarray(np.broadcast_to(
        b1c[:, :, None, None], (128, 4, 8, BL)).copy())

    common = {
        "adj": _f32(adjacency), "eye": np.eye(N, dtype=np.float32),
        "w1d": _bf16(w1d), "w2d": _bf16(w2d), "foldm": foldm,
        "g1r": g1rr, "b1r": b1rr, "g2k": np.ascontiguousarray(g2kk),
        "b2k": np.ascontiguousarray(b2kk), "fm2": fm2,
        "wih0": _bf16(wih0), "wih0b": _bf16(wih0b), "whh0T": _bf16(whh0T),
        "wih1T": _bf16(wih1T), "whh1T": _bf16(whh1T),
        "b0c": b0c, "b1t8": b1t8,
        "fc1T": _bf16(_f32(fc1_w).T), "fc1b": _f32(fc1_b).reshape(-1, 1),
        "fc2T": _bf16(_f32(fc2_w).T), "fc2b": _f32(fc2_b).reshape(-1, 1),
    }
    in_maps = []
    for c in range(NCORES):
        m = dict(common)
        m["xa"] = _bf16(xa_full[:, :, c * BL:(c + 1) * BL, :])
        in_maps.append(m)
    return in_maps


_CACHE = {}


def kernel(**inputs):
    global LAST_EXEC_NS
    debug = bool(int(os.environ.get("STGCN_DEBUG", "0")))
    key = ("dbg" if debug else "std")
    if key not in _CACHE:
        _CACHE[key] = _build(debug=debug)
    nc = _CACHE[key]
    in_maps = _pack_inputs(**inputs)
    kw = {}
    tdir = os.environ.get("STGCN_TRACE_DIR")
    if tdir:
        kw["tmpdir"] = tdir
    res = run_bass_kernel_spmd(nc, in_maps, core_ids=list(range(NCORES)), **kw)
    LAST_EXEC_NS = res.exec_time_ns
    if debug:
        kernel.debug_results = res.results
    out = np.zeros((B, NC_OUT), np.float32)
    for c in range(NCORES):
        out[c * BL:(c + 1) * BL, :] = np.asarray(res.results[c]["y"], np.float32).T
    return out

